# revision 27
# baseline (speedup 1.0000x reference)
"""Trainium2 Bass kernel for nn_DeformConvNet (deformable conv block).

Per-core pipeline (batch-parallel, 1 image per core, 8 cores):
  1. conv1 (C->2C, 3x3) on PE in 16-row strips; the offset-channel
     deinterleave (quirky reshape in the reference) is folded into the weight
     layout (even channels -> par0, odd -> par1) and the PSUM eviction
     (stride-2 reads) so offy/offx land contiguous per 8-row sampling strip.
  2. Deformable bilinear sample in clamp-segment form:
       S_d(i,j) = x(i-2+d, j-2) + sum_k Dx(i-2+d, j-2+k) * cx_k(i,j)
       x_off    = sum_d ty_d(i,j) * S_d(i,j)
     with Dx the horizontal difference image, cx_k = clamp01(offx+2-k)
     (exact linear interpolation; window +-2 exact while |offset| < 2),
     ty via clamp differences (ty_d = P_{d-1} - P_d with P_k = clamp01).
     Weights run as 4x-rate tensor_scalar ops; the H/V adds
     accumulate on the PE via identity matmuls (PSUM) for a subset of the
     5 row-shifts, the rest chain on DVE/Pool.
  3. conv2 (C->PL, 3x3) on PE; bias+relu fused in the PSUM eviction with
     accum_out producing the BN sums for free; Square+accum for sumsq.
  4. BN training stats: tiny [128,2] AllReduce across the 8 cores, then
     y*a+b split across ACT/DVE.
"""

import sys
import numpy as np

for _p in ("/opt/trn_rl_repo",):
    if _p not in sys.path:
        sys.path.insert(0, _p)

import concourse.bass as bass
import concourse.bacc as bacc
import concourse.mybir as mybir
import concourse.tile as tile
from concourse.bass_utils import run_bass_kernel_spmd

F32 = mybir.dt.float32
F16 = mybir.dt.float16
I16 = mybir.dt.int16
AL = mybir.AluOpType
AF = mybir.ActivationFunctionType

B, C, H, W = 8, 128, 128, 128
PL = 128
HP, WP = H + 4, W + 4      # pad-2 image for sampling window
NCORES = 8
EPS = 1e-5
NTOT = float(B * H * W)

TR = 8                     # sampling strip rows
NT = H // TR               # 16 sampling strips
SR = 16                    # conv1 strip rows
NS = H // SR               # 8 conv1 strips

# ---- tuning knobs (env-overridable for sweeps) ----
import os as _os
PE_APPS_EARLY = tuple(int(c) for c in _os.environ.get("K_PE_EARLY", "01234"))
PE_APPS_LATE = tuple(int(c) for c in _os.environ.get("K_PE_LATE", "01234"))
POOL_FRAC = float(_os.environ.get("K_POOL_FRAC", "0.10"))
C1EV_POOL = _os.environ.get("K_C1EV", "act") == "pool"
XOEV_POOL = _os.environ.get("K_XOEV", "act") == "pool"
POOL_M = int(_os.environ.get("K_POOL_M", "0"))     # m-mults per strip on Pool
OUT_F16 = _os.environ.get("K_OUT_F16", "0") == "1"
TY_POOL = _os.environ.get("K_TY_POOL", "0") == "1"
DXL_POOL = _os.environ.get("K_DXL_POOL", "0") == "1"
HEAD_CAST = int(_os.environ.get("K_HEAD_CAST", "0"))
DXL_BUFS = int(_os.environ.get("K_DXL_BUFS", "2"))
H_BUFS = int(_os.environ.get("K_H_BUFS", "2"))
VC2_BUFS = int(_os.environ.get("K_VC2_BUFS", "2"))
M_BUFS = int(_os.environ.get("K_M_BUFS", "2"))
SD_BUFS = int(_os.environ.get("K_SD_BUFS", "2"))
OFF_BUFS = int(_os.environ.get("K_OFF_BUFS", "4"))


def _emit(tc):
    nc = tc.nc
    x_in = nc.declare_dram_parameter("x", [C, H * W], F32, isOutput=False)
    woff_in = nc.declare_dram_parameter("w_off", [C, 18 * C], F32, isOutput=False)
    wconv_in = nc.declare_dram_parameter("w_conv", [C, 9 * PL], F32, isOutput=False)
    b_in = nc.declare_dram_parameter("b_conv", [PL, 3], F32, isOutput=False)
    out_o = nc.declare_dram_parameter("out", [PL, H * W], F32, isOutput=True)

    # Bresenham-style DVE/Pool round robin for TT ops
    rr = {"acc": 0.0}

    def eng_tt():
        rr["acc"] += POOL_FRAC
        if rr["acc"] >= 1.0:
            rr["acc"] -= 1.0
            return nc.gpsimd
        return nc.vector

    with (
        tc.tile_pool(name="const", bufs=1) as const,
        tc.tile_pool(name="dram", bufs=1, space="DRAM") as dram,
        tc.tile_pool(name="offp", bufs=OFF_BUFS) as offp,
        tc.tile_pool(name="wts", bufs=2) as wts,
        tc.tile_pool(name="slab", bufs=2) as slab,
        tc.tile_pool(name="ps", bufs=1, space="PSUM") as psp,
    ):
        # ---------------- loads / constants ----------------
        x16 = const.tile([C, HP * WP], F16)
        x3 = x16[:].rearrange("p (h w) -> p h w", w=WP)
        # zero the pad ring (2 wide); interior filled by DMA
        nc.vector.memset(x3[:, 0:2, :], 0.0)
        nc.vector.memset(x3[:, HP - 2:HP, :], 0.0)
        nc.vector.memset(x3[:, 2:2 + H, 0:2], 0.0)
        nc.vector.memset(x3[:, 2:2 + H, WP - 2:WP], 0.0)
        w1all = const.tile([C, 18 * C], F16)
        nc.gpsimd.dma_start(out=w1all[:], in_=woff_in[:])
        w1 = {(par, uv): w1all[:, (par * 9 + uv) * C:(par * 9 + uv + 1) * C]
              for par in range(2) for uv in range(9)}
        w2all = const.tile([C, 9 * PL], F16)
        nc.gpsimd.dma_start(out=w2all[:], in_=wconv_in[:])
        w2 = [w2all[:, uv * PL:(uv + 1) * PL] for uv in range(9)]

        bgb = const.tile([PL, 3], F32)
        nc.sync.dma_start(out=bgb[:], in_=b_in[:])

        for c16 in range(16):
            if c16 < HEAD_CAST:
                nc.gpsimd.dma_start(
                    out=x3[:, 2 + 8 * c16: 2 + 8 * (c16 + 1), 2:2 + W],
                    in_=x_in[:, 1024 * c16: 1024 * (c16 + 1)].rearrange(
                        "p (r w) -> p r w", w=W))
                continue
            xstg = slab.tile([C, 1024], F32, name=f"xstg{c16}", tag="o32",
                             bufs=2)
            nc.sync.dma_start(out=xstg[:],
                              in_=x_in[:, 1024 * c16: 1024 * (c16 + 1)])
            nc.scalar.activation(
                out=x3[:, 2 + 8 * c16: 2 + 8 * (c16 + 1), 2:2 + W],
                in_=xstg[:].rearrange("p (r w) -> p r w", w=W), func=AF.Copy)
        bias_t = bgb[:, 0:1]
        gamma_t = bgb[:, 1:2]
        beta_t = bgb[:, 2:3]

        # identity weights (f16) for PE pass-through adds
        coli = const.tile([C, C], I16)
        nc.gpsimd.iota(coli[:], pattern=[[1, C]], base=0, channel_multiplier=0)
        rowi = const.tile([C, 1], I16)
        nc.gpsimd.iota(rowi[:], pattern=[[0, 1]], base=0, channel_multiplier=1)
        colf = const.tile([C, C], F16)
        nc.vector.tensor_scalar(out=colf[:], in0=coli[:], scalar1=1.0,
                                scalar2=0.0, op0=AL.mult, op1=AL.add)
        rowf = const.tile([C, 1], F16)
        nc.vector.tensor_scalar(out=rowf[:], in0=rowi[:], scalar1=1.0,
                                scalar2=0.0, op0=AL.mult, op1=AL.add)
        ident = const.tile([C, C], F16)
        nc.vector.tensor_tensor(out=ident[:], in0=colf[:],
                                in1=rowf[:].broadcast_to((C, C)), op=AL.is_equal)

        # conv2 input: sampled image, pad-1
        xoffp = const.tile([C, (H + 2) * (W + 2)], F16)
        xo3 = xoffp[:].rearrange("p (h w) -> p h w", w=W + 2)
        nc.vector.memset(xo3[:, 0:1, :], 0.0)
        nc.vector.memset(xo3[:, H + 1:H + 2, :], 0.0)
        nc.vector.memset(xo3[:, 1:1 + H, 0:1], 0.0)
        nc.vector.memset(xo3[:, 1:1 + H, W + 1:W + 2], 0.0)

        sum2 = const.tile([PL, 2 * NT], F32)
        ssq = const.tile([PL, NT], F32)

        # first few y strips (no free xoffp rows yet) live here
        ybuf = const.tile([PL, 3 * TR * W], F16)

        # offset strips produced by conv1 evictions
        offy_reg, offx_reg = {}, {}

        def get_off(t):
            if t not in offy_reg:
                offy_reg[t] = offp.tile([C, TR * W], F16, name=f"oy{t}", tag="oy")
                offx_reg[t] = offp.tile([C, TR * W], F16, name=f"ox{t}", tag="ox")
            return offy_reg[t], offx_reg[t]

        # ---------------- conv1 strip (16 rows) ----------------
        def conv1_strip(s):
            for b4 in range(4):
                r = 16 * s + 4 * b4      # first conv-pixel row of the block
                for par in range(2):
                    ps = psp.tile([C, 512], F32, tag="c1", bufs=2)
                    for uv in range(9):
                        du, dv = uv // 3 - 1, uv % 3 - 1
                        rhs = x3[:, 2 + r + du: 2 + r + du + 4, 2 + dv: 130 + dv]
                        nc.tensor.matmul(ps[:], lhsT=w1[(par, uv)], rhs=rhs,
                                         start=(uv == 0), stop=(uv == 8))
                    t = s + 8 * par
                    oy, ox = get_off(t)
                    if C1EV_POOL:
                        nc.gpsimd.tensor_copy(out=oy[:, 256 * b4: 256 * b4 + 256],
                                              in_=ps[:, 0:512:2])
                        nc.gpsimd.tensor_copy(out=ox[:, 256 * b4: 256 * b4 + 256],
                                              in_=ps[:, 1:512:2])
                    else:
                        nc.scalar.activation(out=oy[:, 256 * b4: 256 * b4 + 256],
                                             in_=ps[:, 0:512:2], func=AF.Copy)
                        nc.scalar.activation(out=ox[:, 256 * b4: 256 * b4 + 256],
                                             in_=ps[:, 1:512:2], func=AF.Copy)

        # ---------------- sampling ----------------
        cx_reg, nty_reg, sd_reg = {}, {}, {}

        def samp_weights_x(t):
            _, ox = get_off(t)
            ox3 = ox[:].rearrange("p (r w) -> p r w", w=W)
            for col, bound, op in ((0, 0.0, AL.max), (1, -1.0, AL.max),
                                   (W - 2, 1.0, AL.min), (W - 1, 0.0, AL.min)):
                nc.vector.tensor_scalar(out=ox3[:, :, col:col + 1],
                                        in0=ox3[:, :, col:col + 1],
                                        scalar1=bound, scalar2=0.0,
                                        op0=op, op1=AL.add)
            cxa = wts.tile([C, 4 * TR * W], F16, name=f"cx{t}", tag="cx")
            NS1 = TR * W
            for k in range(4):
                cx = cxa[:, k * NS1:(k + 1) * NS1]
                nc.vector.tensor_scalar(out=cx, in0=ox[:],
                                        scalar1=float(2 - k), scalar2=0.0,
                                        op0=AL.add, op1=AL.max)
                nc.vector.tensor_scalar(out=cx, in0=cx,
                                        scalar1=1.0, scalar2=0.0,
                                        op0=AL.min, op1=AL.add)
            cx_reg[t] = cxa

        def samp_weights_y(t):
            oy, _ = get_off(t)
            oy3 = oy[:].rearrange("p (r w) -> p r w", w=W)
            # coordinate clamps only matter at image edges
            if t == 0:
                nc.vector.tensor_scalar(out=oy3[:, 0:1, :], in0=oy3[:, 0:1, :],
                                        scalar1=0.0, scalar2=0.0,
                                        op0=AL.max, op1=AL.add)
                nc.vector.tensor_scalar(out=oy3[:, 1:2, :], in0=oy3[:, 1:2, :],
                                        scalar1=-1.0, scalar2=0.0,
                                        op0=AL.max, op1=AL.add)
            if t == NT - 1:
                nc.vector.tensor_scalar(out=oy3[:, TR - 2:TR - 1, :],
                                        in0=oy3[:, TR - 2:TR - 1, :],
                                        scalar1=1.0, scalar2=0.0,
                                        op0=AL.min, op1=AL.add)
                nc.vector.tensor_scalar(out=oy3[:, TR - 1:TR, :],
                                        in0=oy3[:, TR - 1:TR, :],
                                        scalar1=0.0, scalar2=0.0,
                                        op0=AL.min, op1=AL.add)
            # vertical tents via clamp differences, in one slab:
            #   slot k+1 <- P_k = clamp01(offy + 2 - k); ty_0 = 1 - P_0,
            #   slot d <- ty_d = P_{d-1} - P_d (d=1..3), slot 4 = P_3 = ty_4
            NS1 = TR * W
            tya = wts.tile([C, 5 * NS1], F16, name=f"ty{t}", tag="ty")
            for k in range(4):
                p = tya[:, (k + 1) * NS1:(k + 2) * NS1]
                nc.vector.tensor_scalar(out=p, in0=oy[:],
                                        scalar1=float(2 - k), scalar2=0.0,
                                        op0=AL.add, op1=AL.max)
                nc.vector.tensor_scalar(out=p, in0=p,
                                        scalar1=1.0, scalar2=0.0,
                                        op0=AL.min, op1=AL.add)
            nc.vector.tensor_scalar(out=tya[:, 0:NS1],
                                    in0=tya[:, NS1:2 * NS1],
                                    scalar1=-1.0, scalar2=1.0,
                                    op0=AL.mult, op1=AL.add)
            # ty_1..3 in one overlapping-streams op (reads stay ahead of writes)
            nc.vector.tensor_tensor(out=tya[:, NS1:4 * NS1],
                                    in0=tya[:, NS1:4 * NS1],
                                    in1=tya[:, 2 * NS1:5 * NS1], op=AL.subtract)
            nty_reg[t] = tya

        dxl_reg = {}

        def prefetch_dx(t):
            r0 = TR * t
            dxl = slab.tile([C, 12 * 131], F16, name=f"dxl{t}", tag="dxl",
                            bufs=DXL_BUFS)
            dx3 = dxl[:].rearrange("p (r w) -> p r w", w=131)
            dxeng = nc.gpsimd if DXL_POOL else eng_tt()
            dxeng.tensor_tensor(out=dx3[:, :, :],
                                in0=x3[:, r0:r0 + 12, 1:132],
                                in1=x3[:, r0:r0 + 12, 0:131], op=AL.subtract)
            dxl_reg[t] = dxl

        def samp_H(t, pe_apps):
            r0 = TR * t
            NS1 = TR * W
            cxa = cx_reg[t]
            cxv = cxa[:].rearrange("p (k r w) -> p k r w", k=4, w=W)
            dxl = dxl_reg.pop(t)
            dxt = dxl.tensor
            sda = slab.tile([C, 5 * NS1], F16, name=f"sd{t}", tag="sd",
                            bufs=SD_BUFS)
            for d in range(5):
                sd = sda[:, d * NS1:(d + 1) * NS1]
                # one fused mult: m_d[k, r, j] = Dx[r0-2+d+r, j-2+k] * cx_k(r, j)
                md = slab.tile([C, 4 * NS1], F16, name=f"m{t}_{d}", tag="m",
                               bufs=M_BUFS)
                dx_slab = bass.AP(dxt, dxl.offset + 131 * d,
                                  [[dxt.shape[1], C], [1, 4], [131, TR], [1, W]])
                eng_tt().tensor_tensor(
                    out=md[:].rearrange("p (k r w) -> p k r w", k=4, w=W),
                    in0=dx_slab, in1=cxv, op=AL.mult)
                base = x3[:, r0 + d: r0 + d + 8, 0:W]
                if d in pe_apps:
                    ps = psp.tile([C, 1024], F32, tag="h", bufs=H_BUFS)
                    for h2 in range(2):
                        bh = x3[:, r0 + 4 * h2 + d: r0 + 4 * h2 + d + 4, 0:W]
                        pp = ps[:, 512 * h2: 512 * h2 + 512]
                        nc.tensor.matmul(pp, lhsT=ident[:], rhs=bh,
                                         start=True, stop=False)
                        for k in range(4):
                            nc.tensor.matmul(
                                pp, lhsT=ident[:],
                                rhs=md[:, k * NS1 + 512 * h2:
                                       k * NS1 + 512 * h2 + 512],
                                start=False, stop=(k == 3))
                    nc.scalar.activation(out=sd, in_=ps[:], func=AF.Copy)
                else:
                    sd3 = sd.rearrange("p (r w) -> p r w", w=W)
                    eng_tt().tensor_tensor(
                        out=sd3, in0=base,
                        in1=md[:, 0:NS1].rearrange("p (r w) -> p r w", w=W),
                        op=AL.add)
                    for k in range(1, 4):
                        eng_tt().tensor_tensor(
                            out=sd, in0=sd, in1=md[:, k * NS1:(k + 1) * NS1],
                            op=AL.add)
            sd_reg[t] = sda

        def samp_V(t):
            r0 = TR * t
            NS1 = TR * W
            tya = nty_reg[t]
            sda = sd_reg[t]
            vda = slab.tile([C, 5 * NS1], F16, name=f"vd{t}", tag="vd", bufs=2)
            eng_tt().tensor_tensor(out=vda[:], in0=tya[:], in1=sda[:],
                                   op=AL.mult)
            for h2 in range(2):
                pv = psp.tile([C, 512], F32, tag="vc2", bufs=VC2_BUFS)
                for d in range(5):
                    nc.tensor.matmul(pv[:], lhsT=ident[:],
                                     rhs=vda[:, d * NS1 + 512 * h2:
                                             d * NS1 + 512 * h2 + 512],
                                     start=(d == 0), stop=(d == 4))
                if XOEV_POOL:
                    nc.gpsimd.tensor_copy(
                        out=xo3[:, 1 + r0 + 4 * h2: 1 + r0 + 4 * h2 + 4, 1:1 + W],
                        in_=pv[:].rearrange("p (r w) -> p r w", w=W))
                else:
                    nc.scalar.activation(
                        out=xo3[:, 1 + r0 + 4 * h2: 1 + r0 + 4 * h2 + 4, 1:1 + W],
                        in_=pv[:].rearrange("p (r w) -> p r w", w=W), func=AF.Copy)

        # ---------------- conv2 strip (8 rows) ----------------
        y_slot = {}          # t -> AP view of the stored y strip
        xo_free: set = set()  # xoffp row-blocks whose conv2 readers are done
        ybuf_used = [0]

        def y_dst(t):
            # block b (xo3 rows 8b..8b+8) is read by conv2(b-1) and conv2(b)
            for b in sorted(xo_free):
                xo_free.discard(b)
                return xo3[:, TR * b: TR * b + TR, 1:1 + W]
            i = ybuf_used[0]
            ybuf_used[0] += 1
            assert i < 3, "ybuf overflow"
            return ybuf[:, i * TR * W:(i + 1) * TR * W].rearrange(
                "p (r w) -> p r w", w=W)

        def conv2_strip(t):
            r0 = TR * t
            for b in range(NT):
                bdeps = {u for u in (b - 1, b) if 0 <= u < NT}
                if b not in xo_free and bdeps <= (c2_emitted | {t})                         and b not in y_blocks:
                    xo_free.add(b)
                    y_blocks.add(b)
            ydst = y_dst(t)
            y_slot[t] = ydst
            for h2 in range(2):
                ps = psp.tile([C, 512], F32, tag="vc2", bufs=VC2_BUFS)
                rq = r0 + 4 * h2
                for uv in range(9):
                    du, dv = uv // 3 - 1, uv % 3 - 1
                    rhs = xo3[:, 1 + rq + du: 1 + rq + du + 4, 1 + dv: 1 + dv + W]
                    nc.tensor.matmul(ps[:], lhsT=w2[uv], rhs=rhs,
                                     start=(uv == 0), stop=(uv == 8))
                nc.scalar.activation(out=ydst[:, 4 * h2: 4 * h2 + 4, :],
                                     in_=ps[:].rearrange("p (r w) -> p r w", w=W),
                                     func=AF.Relu, bias=bias_t, scale=1.0,
                                     accum_out=sum2[:, 2 * t + h2: 2 * t + h2 + 1])
            sq = slab.tile([PL, TR * W], F16, name=f"sq{t}", tag="o32", bufs=2)
            nc.scalar.activation(out=sq[:].rearrange("p (r w) -> p r w", w=W),
                                 in_=ydst, func=AF.Square,
                                 accum_out=ssq[:, t:t + 1])

        # ---------------- schedule ----------------
        order = [t for s in range(NS) for t in (s + 8, s)]
        y_blocks: set = set()
        deps = {t: {u for u in (t - 1, t, t + 1) if 0 <= u < NT} for t in range(NT)}
        v_done: set = set()
        c2_emitted: set = set()

        def emit_ready_conv2():
            for tt2 in range(NT):
                if tt2 not in c2_emitted and deps[tt2] <= v_done:
                    conv2_strip(tt2)
                    c2_emitted.add(tt2)

        conv1_strip(0)
        conv1_strip(1)
        prefetch_dx(order[0])
        prefetch_dx(order[1])
        prev = None
        for n, t in enumerate(order):
            if n % 2 == 0 and n // 2 + 2 < NS:
                conv1_strip(n // 2 + 2)
            if n + 2 < len(order):
                prefetch_dx(order[n + 2])
            samp_weights_x(t)
            samp_H(t, PE_APPS_EARLY if n < 8 else PE_APPS_LATE)
            samp_weights_y(t)
            if prev is not None:
                samp_V(prev)
                v_done.add(prev)
                emit_ready_conv2()
            prev = t
        samp_V(prev)
        v_done.add(prev)
        emit_ready_conv2()

        # ---------------- stats + collective + normalize ----------------
        st2 = const.tile([PL, 2], F32)
        nc.vector.tensor_reduce(out=st2[:, 0:1], in_=sum2[:],
                                axis=mybir.AxisListType.X, op=AL.add)
        nc.vector.tensor_reduce(out=st2[:, 1:2], in_=ssq[:],
                                axis=mybir.AxisListType.X, op=AL.add)
        cc_in = dram.tile([PL, 2], F32)
        cc_out = dram.tile([PL, 2], F32)
        nc.gpsimd.dma_start(out=cc_in[:], in_=st2[:])
        nc.gpsimd.collective_compute(
            "AllReduce", AL.add,
            replica_groups=[list(range(NCORES))],
            ins=[cc_in.opt()], outs=[cc_out.opt()],
        )
        stg = const.tile([PL, 2], F32)
        nc.gpsimd.dma_start(out=stg[:], in_=cc_out[:])

        mean = const.tile([PL, 1], F32)
        nc.vector.tensor_scalar(out=mean[:], in0=stg[:, 0:1], scalar1=1.0 / NTOT,
                                scalar2=0.0, op0=AL.mult, op1=AL.add)
        ex2 = const.tile([PL, 1], F32)
        nc.vector.tensor_scalar(out=ex2[:], in0=stg[:, 1:2], scalar1=1.0 / NTOT,
                                scalar2=0.0, op0=AL.mult, op1=AL.add)
        var = const.tile([PL, 1], F32)
        nc.vector.tensor_tensor(out=var[:], in0=mean[:], in1=mean[:], op=AL.mult)
        nc.vector.tensor_tensor(out=var[:], in0=ex2[:], in1=var[:], op=AL.subtract)
        epst = const.tile([PL, 1], F32)
        nc.gpsimd.memset(epst[:], EPS)
        stdv = const.tile([PL, 1], F32)
        nc.scalar.activation(out=stdv[:], in_=var[:], func=AF.Sqrt, bias=epst[:])
        rstd = const.tile([PL, 1], F32)
        nc.vector.reciprocal(rstd[:], stdv[:])
        avec = const.tile([PL, 1], F32)
        nc.vector.tensor_tensor(out=avec[:], in0=gamma_t, in1=rstd[:], op=AL.mult)
        bvec = const.tile([PL, 1], F32)
        nc.vector.tensor_tensor(out=bvec[:], in0=avec[:], in1=mean[:], op=AL.mult)
        nc.vector.tensor_tensor(out=bvec[:], in0=beta_t, in1=bvec[:],
                                op=AL.subtract)

        for t in range(NT):
            r0 = TR * t
            ysrc = y_slot[t]
            if OUT_F16:
                ot = slab.tile([PL, TR * W], F16, name=f"o16{t}", tag="o32", bufs=2)
            else:
                ot = slab.tile([PL, TR * W], F32, name=f"o32{t}", tag="o32",
                               bufs=2)
            o3v = ot[:].rearrange("p (r w) -> p r w", w=W)
            if t % 2 == 0:
                nc.scalar.activation(out=o3v, in_=ysrc, func=AF.Identity,
                                     bias=bvec[:], scale=avec[:])
            else:
                nc.vector.tensor_scalar(out=o3v, in0=ysrc, scalar1=avec[:],
                                        scalar2=bvec[:], op0=AL.mult, op1=AL.add)
            if OUT_F16:
                nc.gpsimd.dma_start(out=out_o[:, r0 * W:(r0 + TR) * W],
                                    in_=ot[:])
            else:
                nc.sync.dma_start(out=out_o[:, r0 * W:(r0 + TR) * W],
                                  in_=ot[:])


_NC_CACHE = None


def _get_nc():
    global _NC_CACHE
    if _NC_CACHE is None:
        nc = bacc.Bacc("TRN2", target_bir_lowering=False, debug=False,
                       num_devices=NCORES)
        with tile.TileContext(nc) as tc:
            _emit(tc)
        nc.compile()
        _NC_CACHE = nc
    return _NC_CACHE


def kernel(**inputs):
    x = np.ascontiguousarray(np.asarray(inputs["x"], dtype=np.float32))
    w_off = np.asarray(inputs["w_off"], dtype=np.float32).reshape(C, 2, C, 9)
    w_off_t = np.ascontiguousarray(
        w_off.transpose(2, 1, 3, 0).reshape(C, 18 * C))
    w_conv = np.asarray(inputs["w_conv"], dtype=np.float32).reshape(PL, C, 9)
    w_conv_t = np.ascontiguousarray(
        w_conv.transpose(1, 2, 0).reshape(C, 9 * PL))
    bgb = np.stack([
        np.asarray(inputs["b_conv"], np.float32).reshape(PL),
        np.asarray(inputs["gamma"], np.float32).reshape(PL),
        np.asarray(inputs["beta"], np.float32).reshape(PL),
    ], axis=1)

    nc = _get_nc()
    global LAST_RESULTS
    in_maps = [
        {
            "x": np.ascontiguousarray(x[b].reshape(C, H * W)),
            "w_off": w_off_t,
            "w_conv": w_conv_t,
            "b_conv": np.ascontiguousarray(bgb),
        }
        for b in range(B)
    ]
    res = run_bass_kernel_spmd(nc, in_maps, core_ids=list(range(NCORES)))
    LAST_RESULTS = res
    out = np.stack([res.results[b]["out"].reshape(PL, H, W) for b in range(B)])
    return out.astype(np.float32)


LAST_RESULTS = None


if __name__ == "__main__":
    rng = np.random.default_rng(0)
    ins = {
        "x": rng.normal(size=(B, C, H, W)).astype(np.float32),
        "w_off": (rng.normal(size=(2 * C, C, 3, 3)) * 0.01).astype(np.float32),
        "w_conv": (rng.normal(size=(PL, C, 3, 3)) * 0.05).astype(np.float32),
        "b_conv": (rng.normal(size=(PL,)) * 0.01).astype(np.float32),
        "gamma": np.ones((PL,), np.float32),
        "beta": np.zeros((PL,), np.float32),
    }
    out = kernel(**ins)
    print("out", out.shape, out.dtype, float(np.abs(out).max()))


# revision 28
# speedup vs baseline: 1.0340x; 1.0340x over previous
"""Trainium2 Bass kernel for nn_DeformConvNet (deformable conv block).

Per-core pipeline (batch-parallel, 1 image per core, 8 cores):
  1. conv1 (C->2C, 3x3) on PE in 16-row strips; the offset-channel
     deinterleave (quirky reshape in the reference) is folded into the weight
     layout (even channels -> par0, odd -> par1) and the PSUM eviction
     (stride-2 reads) so offy/offx land contiguous per 8-row sampling strip.
  2. Deformable bilinear sample in clamp-segment form:
       S_d(i,j) = x(i-2+d, j-2) + sum_k Dx(i-2+d, j-2+k) * cx_k(i,j)
       x_off    = sum_d ty_d(i,j) * S_d(i,j)
     with Dx the horizontal difference image, cx_k = clamp01(offx+2-k)
     (exact linear interpolation; window +-2 exact while |offset| < 2),
     ty via clamp differences (ty_d = P_{d-1} - P_d with P_k = clamp01).
     Weights run as 4x-rate tensor_scalar ops; the H/V adds
     accumulate on the PE via identity matmuls (PSUM) for a subset of the
     5 row-shifts, the rest chain on DVE/Pool.
  3. conv2 (C->PL, 3x3) on PE; bias+relu fused in the PSUM eviction with
     accum_out producing the BN sums for free; Square+accum for sumsq.
  4. BN training stats: tiny [128,2] AllReduce across the 8 cores, then
     y*a+b split across ACT/DVE.
"""

import sys
import numpy as np

for _p in ("/opt/trn_rl_repo",):
    if _p not in sys.path:
        sys.path.insert(0, _p)

import concourse.bass as bass
import concourse.bacc as bacc
import concourse.mybir as mybir
import concourse.tile as tile
from concourse.bass_utils import run_bass_kernel_spmd

F32 = mybir.dt.float32
F16 = mybir.dt.float16
I16 = mybir.dt.int16
AL = mybir.AluOpType
AF = mybir.ActivationFunctionType

B, C, H, W = 8, 128, 128, 128
PL = 128
HP, WP = H + 4, W + 4      # pad-2 image for sampling window
NCORES = 8
EPS = 1e-5
NTOT = float(B * H * W)

TR = 8                     # sampling strip rows
NT = H // TR               # 16 sampling strips
SR = 16                    # conv1 strip rows
NS = H // SR               # 8 conv1 strips

# ---- tuning knobs (env-overridable for sweeps) ----
import os as _os
PE_APPS_EARLY = tuple(int(c) for c in _os.environ.get("K_PE_EARLY", "01234"))
PE_APPS_LATE = tuple(int(c) for c in _os.environ.get("K_PE_LATE", "01234"))
POOL_FRAC = float(_os.environ.get("K_POOL_FRAC", "0.10"))
C1EV_POOL = _os.environ.get("K_C1EV", "act") == "pool"
XOEV_POOL = _os.environ.get("K_XOEV", "act") == "pool"
POOL_M = int(_os.environ.get("K_POOL_M", "0"))     # m-mults per strip on Pool
OUT_F16 = _os.environ.get("K_OUT_F16", "0") == "1"
TY_POOL = _os.environ.get("K_TY_POOL", "0") == "1"
DXL_POOL = _os.environ.get("K_DXL_POOL", "0") == "1"
HEAD_CAST = int(_os.environ.get("K_HEAD_CAST", "0"))
DXL_BUFS = int(_os.environ.get("K_DXL_BUFS", "2"))
H_BUFS = int(_os.environ.get("K_H_BUFS", "2"))
VC2_BUFS = int(_os.environ.get("K_VC2_BUFS", "2"))
M_BUFS = int(_os.environ.get("K_M_BUFS", "2"))
SD_BUFS = int(_os.environ.get("K_SD_BUFS", "2"))
OFF_BUFS = int(_os.environ.get("K_OFF_BUFS", "4"))


def _emit(tc):
    nc = tc.nc
    x_in = nc.declare_dram_parameter("x", [C, H * W], F32, isOutput=False)
    woff_in = nc.declare_dram_parameter("w_off", [C, 18 * C], F32, isOutput=False)
    wconv_in = nc.declare_dram_parameter("w_conv", [C, 9 * PL], F32, isOutput=False)
    b_in = nc.declare_dram_parameter("b_conv", [PL, 3], F32, isOutput=False)
    out_o = nc.declare_dram_parameter("out", [PL, H * W], F32, isOutput=True)

    # Bresenham-style DVE/Pool round robin for TT ops
    rr = {"acc": 0.0}

    def eng_tt():
        rr["acc"] += POOL_FRAC
        if rr["acc"] >= 1.0:
            rr["acc"] -= 1.0
            return nc.gpsimd
        return nc.vector

    with (
        tc.tile_pool(name="const", bufs=1) as const,
        tc.tile_pool(name="dram", bufs=1, space="DRAM") as dram,
        tc.tile_pool(name="offp", bufs=OFF_BUFS) as offp,
        tc.tile_pool(name="wts", bufs=2) as wts,
        tc.tile_pool(name="slab", bufs=2) as slab,
        tc.tile_pool(name="ps", bufs=1, space="PSUM") as psp,
    ):
        # ---------------- loads / constants ----------------
        x16 = const.tile([C, HP * WP], F16)
        x3 = x16[:].rearrange("p (h w) -> p h w", w=WP)
        # zero the pad ring (2 wide); interior filled by DMA
        nc.vector.memset(x3[:, 0:2, :], 0.0)
        nc.vector.memset(x3[:, HP - 2:HP, :], 0.0)
        nc.vector.memset(x3[:, 2:2 + H, 0:2], 0.0)
        nc.vector.memset(x3[:, 2:2 + H, WP - 2:WP], 0.0)
        w1all = const.tile([C, 18 * C], F16)
        nc.gpsimd.dma_start(out=w1all[:], in_=woff_in[:])
        w1 = {(par, uv): w1all[:, (par * 9 + uv) * C:(par * 9 + uv + 1) * C]
              for par in range(2) for uv in range(9)}
        w2all = const.tile([C, 9 * PL], F16)
        nc.gpsimd.dma_start(out=w2all[:], in_=wconv_in[:])
        w2 = [w2all[:, uv * PL:(uv + 1) * PL] for uv in range(9)]

        bgb = const.tile([PL, 3], F32)
        nc.sync.dma_start(out=bgb[:], in_=b_in[:])

        for c16 in range(16):
            if c16 < HEAD_CAST:
                nc.gpsimd.dma_start(
                    out=x3[:, 2 + 8 * c16: 2 + 8 * (c16 + 1), 2:2 + W],
                    in_=x_in[:, 1024 * c16: 1024 * (c16 + 1)].rearrange(
                        "p (r w) -> p r w", w=W))
                continue
            xstg = slab.tile([C, 1024], F32, name=f"xstg{c16}", tag="o32",
                             bufs=2)
            nc.sync.dma_start(out=xstg[:],
                              in_=x_in[:, 1024 * c16: 1024 * (c16 + 1)])
            nc.scalar.activation(
                out=x3[:, 2 + 8 * c16: 2 + 8 * (c16 + 1), 2:2 + W],
                in_=xstg[:].rearrange("p (r w) -> p r w", w=W), func=AF.Copy)
        bias_t = bgb[:, 0:1]
        gamma_t = bgb[:, 1:2]
        beta_t = bgb[:, 2:3]

        # identity weights (f16) for PE pass-through adds
        coli = const.tile([C, C], I16)
        nc.gpsimd.iota(coli[:], pattern=[[1, C]], base=0, channel_multiplier=0)
        rowi = const.tile([C, 1], I16)
        nc.gpsimd.iota(rowi[:], pattern=[[0, 1]], base=0, channel_multiplier=1)
        colf = const.tile([C, C], F16)
        nc.vector.tensor_scalar(out=colf[:], in0=coli[:], scalar1=1.0,
                                scalar2=0.0, op0=AL.mult, op1=AL.add)
        rowf = const.tile([C, 1], F16)
        nc.vector.tensor_scalar(out=rowf[:], in0=rowi[:], scalar1=1.0,
                                scalar2=0.0, op0=AL.mult, op1=AL.add)
        ident = const.tile([C, C], F16)
        nc.vector.tensor_tensor(out=ident[:], in0=colf[:],
                                in1=rowf[:].broadcast_to((C, C)), op=AL.is_equal)

        # conv2 input: sampled image, pad-1
        xoffp = const.tile([C, (H + 2) * (W + 2)], F16)
        xo3 = xoffp[:].rearrange("p (h w) -> p h w", w=W + 2)
        nc.vector.memset(xo3[:, 0:1, :], 0.0)
        nc.vector.memset(xo3[:, H + 1:H + 2, :], 0.0)
        nc.vector.memset(xo3[:, 1:1 + H, 0:1], 0.0)
        nc.vector.memset(xo3[:, 1:1 + H, W + 1:W + 2], 0.0)

        sum2 = const.tile([PL, 2 * NT], F32)
        ssq = const.tile([PL, NT], F32)

        # first few y strips (no free xoffp rows yet) live here
        ybuf = const.tile([PL, 3 * TR * W], F16)

        # offset strips produced by conv1 evictions
        offy_reg, offx_reg = {}, {}

        def get_off(t):
            if t not in offy_reg:
                offy_reg[t] = offp.tile([C, TR * W], F16, name=f"oy{t}", tag="oy")
                offx_reg[t] = offp.tile([C, TR * W], F16, name=f"ox{t}", tag="ox")
            return offy_reg[t], offx_reg[t]

        # ---------------- conv1 strip (16 rows) ----------------
        def conv1_half(s, half):
            for b4 in (2 * half, 2 * half + 1):
                r = 16 * s + 4 * b4      # first conv-pixel row of the block
                for par in range(2):
                    ps = psp.tile([C, 512], F32, tag="c1", bufs=2)
                    for uv in range(9):
                        du, dv = uv // 3 - 1, uv % 3 - 1
                        rhs = x3[:, 2 + r + du: 2 + r + du + 4, 2 + dv: 130 + dv]
                        nc.tensor.matmul(ps[:], lhsT=w1[(par, uv)], rhs=rhs,
                                         start=(uv == 0), stop=(uv == 8))
                    t = s + 8 * par
                    oy, ox = get_off(t)
                    if C1EV_POOL:
                        nc.gpsimd.tensor_copy(out=oy[:, 256 * b4: 256 * b4 + 256],
                                              in_=ps[:, 0:512:2])
                        nc.gpsimd.tensor_copy(out=ox[:, 256 * b4: 256 * b4 + 256],
                                              in_=ps[:, 1:512:2])
                    else:
                        nc.scalar.activation(out=oy[:, 256 * b4: 256 * b4 + 256],
                                             in_=ps[:, 0:512:2], func=AF.Copy)
                        nc.scalar.activation(out=ox[:, 256 * b4: 256 * b4 + 256],
                                             in_=ps[:, 1:512:2], func=AF.Copy)

        # ---------------- sampling ----------------
        cx_reg, nty_reg, sd_reg = {}, {}, {}

        def samp_weights_x(t):
            _, ox = get_off(t)
            ox3 = ox[:].rearrange("p (r w) -> p r w", w=W)
            for col, bound, op in ((0, 0.0, AL.max), (1, -1.0, AL.max),
                                   (W - 2, 1.0, AL.min), (W - 1, 0.0, AL.min)):
                nc.vector.tensor_scalar(out=ox3[:, :, col:col + 1],
                                        in0=ox3[:, :, col:col + 1],
                                        scalar1=bound, scalar2=0.0,
                                        op0=op, op1=AL.add)
            cxa = wts.tile([C, 4 * TR * W], F16, name=f"cx{t}", tag="cx")
            NS1 = TR * W
            for k in range(4):
                cx = cxa[:, k * NS1:(k + 1) * NS1]
                nc.vector.tensor_scalar(out=cx, in0=ox[:],
                                        scalar1=float(2 - k), scalar2=0.0,
                                        op0=AL.add, op1=AL.max)
                nc.vector.tensor_scalar(out=cx, in0=cx,
                                        scalar1=1.0, scalar2=0.0,
                                        op0=AL.min, op1=AL.add)
            cx_reg[t] = cxa

        def samp_weights_y(t):
            oy, _ = get_off(t)
            oy3 = oy[:].rearrange("p (r w) -> p r w", w=W)
            # coordinate clamps only matter at image edges
            if t == 0:
                nc.vector.tensor_scalar(out=oy3[:, 0:1, :], in0=oy3[:, 0:1, :],
                                        scalar1=0.0, scalar2=0.0,
                                        op0=AL.max, op1=AL.add)
                nc.vector.tensor_scalar(out=oy3[:, 1:2, :], in0=oy3[:, 1:2, :],
                                        scalar1=-1.0, scalar2=0.0,
                                        op0=AL.max, op1=AL.add)
            if t == NT - 1:
                nc.vector.tensor_scalar(out=oy3[:, TR - 2:TR - 1, :],
                                        in0=oy3[:, TR - 2:TR - 1, :],
                                        scalar1=1.0, scalar2=0.0,
                                        op0=AL.min, op1=AL.add)
                nc.vector.tensor_scalar(out=oy3[:, TR - 1:TR, :],
                                        in0=oy3[:, TR - 1:TR, :],
                                        scalar1=0.0, scalar2=0.0,
                                        op0=AL.min, op1=AL.add)
            # vertical tents via clamp differences, in one slab:
            #   slot k+1 <- P_k = clamp01(offy + 2 - k); ty_0 = 1 - P_0,
            #   slot d <- ty_d = P_{d-1} - P_d (d=1..3), slot 4 = P_3 = ty_4
            NS1 = TR * W
            tya = wts.tile([C, 5 * NS1], F16, name=f"ty{t}", tag="ty")
            for k in range(4):
                p = tya[:, (k + 1) * NS1:(k + 2) * NS1]
                nc.vector.tensor_scalar(out=p, in0=oy[:],
                                        scalar1=float(2 - k), scalar2=0.0,
                                        op0=AL.add, op1=AL.max)
                nc.vector.tensor_scalar(out=p, in0=p,
                                        scalar1=1.0, scalar2=0.0,
                                        op0=AL.min, op1=AL.add)
            nc.vector.tensor_scalar(out=tya[:, 0:NS1],
                                    in0=tya[:, NS1:2 * NS1],
                                    scalar1=-1.0, scalar2=1.0,
                                    op0=AL.mult, op1=AL.add)
            # ty_1..3 in one overlapping-streams op (reads stay ahead of writes)
            nc.vector.tensor_tensor(out=tya[:, NS1:4 * NS1],
                                    in0=tya[:, NS1:4 * NS1],
                                    in1=tya[:, 2 * NS1:5 * NS1], op=AL.subtract)
            nty_reg[t] = tya

        dxl_reg = {}

        def prefetch_dx(t):
            r0 = TR * t
            dxl = slab.tile([C, 12 * 131], F16, name=f"dxl{t}", tag="dxl",
                            bufs=DXL_BUFS)
            dx3 = dxl[:].rearrange("p (r w) -> p r w", w=131)
            dxeng = nc.gpsimd if DXL_POOL else eng_tt()
            dxeng.tensor_tensor(out=dx3[:, :, :],
                                in0=x3[:, r0:r0 + 12, 1:132],
                                in1=x3[:, r0:r0 + 12, 0:131], op=AL.subtract)
            dxl_reg[t] = dxl

        def samp_H(t, pe_apps):
            r0 = TR * t
            NS1 = TR * W
            cxa = cx_reg[t]
            cxv = cxa[:].rearrange("p (k r w) -> p k r w", k=4, w=W)
            dxl = dxl_reg.pop(t)
            dxt = dxl.tensor
            sda = slab.tile([C, 5 * NS1], F16, name=f"sd{t}", tag="sd",
                            bufs=SD_BUFS)
            for d in range(5):
                sd = sda[:, d * NS1:(d + 1) * NS1]
                # one fused mult: m_d[k, r, j] = Dx[r0-2+d+r, j-2+k] * cx_k(r, j)
                md = slab.tile([C, 4 * NS1], F16, name=f"m{t}_{d}", tag="m",
                               bufs=M_BUFS)
                dx_slab = bass.AP(dxt, dxl.offset + 131 * d,
                                  [[dxt.shape[1], C], [1, 4], [131, TR], [1, W]])
                eng_tt().tensor_tensor(
                    out=md[:].rearrange("p (k r w) -> p k r w", k=4, w=W),
                    in0=dx_slab, in1=cxv, op=AL.mult)
                base = x3[:, r0 + d: r0 + d + 8, 0:W]
                if d in pe_apps:
                    ps = psp.tile([C, 1024], F32, tag="h", bufs=H_BUFS)
                    for h2 in range(2):
                        bh = x3[:, r0 + 4 * h2 + d: r0 + 4 * h2 + d + 4, 0:W]
                        pp = ps[:, 512 * h2: 512 * h2 + 512]
                        nc.tensor.matmul(pp, lhsT=ident[:], rhs=bh,
                                         start=True, stop=False)
                        for k in range(4):
                            nc.tensor.matmul(
                                pp, lhsT=ident[:],
                                rhs=md[:, k * NS1 + 512 * h2:
                                       k * NS1 + 512 * h2 + 512],
                                start=False, stop=(k == 3))
                    nc.scalar.activation(out=sd, in_=ps[:], func=AF.Copy)
                else:
                    sd3 = sd.rearrange("p (r w) -> p r w", w=W)
                    eng_tt().tensor_tensor(
                        out=sd3, in0=base,
                        in1=md[:, 0:NS1].rearrange("p (r w) -> p r w", w=W),
                        op=AL.add)
                    for k in range(1, 4):
                        eng_tt().tensor_tensor(
                            out=sd, in0=sd, in1=md[:, k * NS1:(k + 1) * NS1],
                            op=AL.add)
            sd_reg[t] = sda

        def samp_V(t):
            r0 = TR * t
            NS1 = TR * W
            tya = nty_reg[t]
            sda = sd_reg[t]
            vda = slab.tile([C, 5 * NS1], F16, name=f"vd{t}", tag="vd", bufs=2)
            eng_tt().tensor_tensor(out=vda[:], in0=tya[:], in1=sda[:],
                                   op=AL.mult)
            for h2 in range(2):
                pv = psp.tile([C, 512], F32, tag="vc2", bufs=VC2_BUFS)
                for d in range(5):
                    nc.tensor.matmul(pv[:], lhsT=ident[:],
                                     rhs=vda[:, d * NS1 + 512 * h2:
                                             d * NS1 + 512 * h2 + 512],
                                     start=(d == 0), stop=(d == 4))
                if XOEV_POOL:
                    nc.gpsimd.tensor_copy(
                        out=xo3[:, 1 + r0 + 4 * h2: 1 + r0 + 4 * h2 + 4, 1:1 + W],
                        in_=pv[:].rearrange("p (r w) -> p r w", w=W))
                else:
                    nc.scalar.activation(
                        out=xo3[:, 1 + r0 + 4 * h2: 1 + r0 + 4 * h2 + 4, 1:1 + W],
                        in_=pv[:].rearrange("p (r w) -> p r w", w=W), func=AF.Copy)

        # ---------------- conv2 strip (8 rows) ----------------
        y_slot = {}          # t -> AP view of the stored y strip
        xo_free: set = set()  # xoffp row-blocks whose conv2 readers are done
        ybuf_used = [0]

        def y_dst(t):
            # block b (xo3 rows 8b..8b+8) is read by conv2(b-1) and conv2(b)
            for b in sorted(xo_free):
                xo_free.discard(b)
                return xo3[:, TR * b: TR * b + TR, 1:1 + W]
            i = ybuf_used[0]
            ybuf_used[0] += 1
            assert i < 3, "ybuf overflow"
            return ybuf[:, i * TR * W:(i + 1) * TR * W].rearrange(
                "p (r w) -> p r w", w=W)

        def conv2_strip(t):
            r0 = TR * t
            for b in range(NT):
                bdeps = {u for u in (b - 1, b) if 0 <= u < NT}
                if b not in xo_free and bdeps <= (c2_emitted | {t})                         and b not in y_blocks:
                    xo_free.add(b)
                    y_blocks.add(b)
            ydst = y_dst(t)
            y_slot[t] = ydst
            for h2 in range(2):
                ps = psp.tile([C, 512], F32, tag="vc2", bufs=VC2_BUFS)
                rq = r0 + 4 * h2
                for uv in range(9):
                    du, dv = uv // 3 - 1, uv % 3 - 1
                    rhs = xo3[:, 1 + rq + du: 1 + rq + du + 4, 1 + dv: 1 + dv + W]
                    nc.tensor.matmul(ps[:], lhsT=w2[uv], rhs=rhs,
                                     start=(uv == 0), stop=(uv == 8))
                nc.scalar.activation(out=ydst[:, 4 * h2: 4 * h2 + 4, :],
                                     in_=ps[:].rearrange("p (r w) -> p r w", w=W),
                                     func=AF.Relu, bias=bias_t, scale=1.0,
                                     accum_out=sum2[:, 2 * t + h2: 2 * t + h2 + 1])
            sq = slab.tile([PL, TR * W], F16, name=f"sq{t}", tag="o32", bufs=2)
            nc.scalar.activation(out=sq[:].rearrange("p (r w) -> p r w", w=W),
                                 in_=ydst, func=AF.Square,
                                 accum_out=ssq[:, t:t + 1])

        # ---------------- schedule ----------------
        order = [t for s in range(NS) for t in (s + 8, s)]
        y_blocks: set = set()
        deps = {t: {u for u in (t - 1, t, t + 1) if 0 <= u < NT} for t in range(NT)}
        v_done: set = set()
        c2_emitted: set = set()

        def emit_ready_conv2():
            for tt2 in range(NT):
                if tt2 not in c2_emitted and deps[tt2] <= v_done:
                    conv2_strip(tt2)
                    c2_emitted.add(tt2)

        for s0 in (0, 1):
            conv1_half(s0, 0)
            conv1_half(s0, 1)
        prefetch_dx(order[0])
        prefetch_dx(order[1])
        prev = None
        for n, t in enumerate(order):
            emit_ready_conv2()
            if prev is not None:
                samp_V(prev)
                v_done.add(prev)
            s_next = n // 2 + 2
            if s_next < NS:
                conv1_half(s_next, n % 2)
            if n + 2 < len(order):
                prefetch_dx(order[n + 2])
            samp_weights_x(t)
            samp_H(t, PE_APPS_EARLY if n < 8 else PE_APPS_LATE)
            samp_weights_y(t)
            prev = t
        samp_V(prev)
        v_done.add(prev)
        emit_ready_conv2()

        # ---------------- stats + collective + normalize ----------------
        st2 = const.tile([PL, 2], F32)
        nc.vector.tensor_reduce(out=st2[:, 0:1], in_=sum2[:],
                                axis=mybir.AxisListType.X, op=AL.add)
        nc.vector.tensor_reduce(out=st2[:, 1:2], in_=ssq[:],
                                axis=mybir.AxisListType.X, op=AL.add)
        cc_in = dram.tile([PL, 2], F32)
        cc_out = dram.tile([PL, 2], F32)
        nc.gpsimd.dma_start(out=cc_in[:], in_=st2[:])
        nc.gpsimd.collective_compute(
            "AllReduce", AL.add,
            replica_groups=[list(range(NCORES))],
            ins=[cc_in.opt()], outs=[cc_out.opt()],
        )
        stg = const.tile([PL, 2], F32)
        nc.gpsimd.dma_start(out=stg[:], in_=cc_out[:])

        mean = const.tile([PL, 1], F32)
        nc.vector.tensor_scalar(out=mean[:], in0=stg[:, 0:1], scalar1=1.0 / NTOT,
                                scalar2=0.0, op0=AL.mult, op1=AL.add)
        ex2 = const.tile([PL, 1], F32)
        nc.vector.tensor_scalar(out=ex2[:], in0=stg[:, 1:2], scalar1=1.0 / NTOT,
                                scalar2=0.0, op0=AL.mult, op1=AL.add)
        var = const.tile([PL, 1], F32)
        nc.vector.tensor_tensor(out=var[:], in0=mean[:], in1=mean[:], op=AL.mult)
        nc.vector.tensor_tensor(out=var[:], in0=ex2[:], in1=var[:], op=AL.subtract)
        epst = const.tile([PL, 1], F32)
        nc.gpsimd.memset(epst[:], EPS)
        stdv = const.tile([PL, 1], F32)
        nc.scalar.activation(out=stdv[:], in_=var[:], func=AF.Sqrt, bias=epst[:])
        rstd = const.tile([PL, 1], F32)
        nc.vector.reciprocal(rstd[:], stdv[:])
        avec = const.tile([PL, 1], F32)
        nc.vector.tensor_tensor(out=avec[:], in0=gamma_t, in1=rstd[:], op=AL.mult)
        bvec = const.tile([PL, 1], F32)
        nc.vector.tensor_tensor(out=bvec[:], in0=avec[:], in1=mean[:], op=AL.mult)
        nc.vector.tensor_tensor(out=bvec[:], in0=beta_t, in1=bvec[:],
                                op=AL.subtract)

        for t in range(NT):
            r0 = TR * t
            ysrc = y_slot[t]
            if OUT_F16:
                ot = slab.tile([PL, TR * W], F16, name=f"o16{t}", tag="o32", bufs=2)
            else:
                ot = slab.tile([PL, TR * W], F32, name=f"o32{t}", tag="o32",
                               bufs=2)
            o3v = ot[:].rearrange("p (r w) -> p r w", w=W)
            if t % 2 == 0:
                nc.scalar.activation(out=o3v, in_=ysrc, func=AF.Identity,
                                     bias=bvec[:], scale=avec[:])
            else:
                nc.vector.tensor_scalar(out=o3v, in0=ysrc, scalar1=avec[:],
                                        scalar2=bvec[:], op0=AL.mult, op1=AL.add)
            if OUT_F16:
                nc.gpsimd.dma_start(out=out_o[:, r0 * W:(r0 + TR) * W],
                                    in_=ot[:])
            else:
                nc.sync.dma_start(out=out_o[:, r0 * W:(r0 + TR) * W],
                                  in_=ot[:])


_NC_CACHE = None


def _get_nc():
    global _NC_CACHE
    if _NC_CACHE is None:
        nc = bacc.Bacc("TRN2", target_bir_lowering=False, debug=False,
                       num_devices=NCORES)
        with tile.TileContext(nc) as tc:
            _emit(tc)
        nc.compile()
        _NC_CACHE = nc
    return _NC_CACHE


def kernel(**inputs):
    x = np.ascontiguousarray(np.asarray(inputs["x"], dtype=np.float32))
    w_off = np.asarray(inputs["w_off"], dtype=np.float32).reshape(C, 2, C, 9)
    w_off_t = np.ascontiguousarray(
        w_off.transpose(2, 1, 3, 0).reshape(C, 18 * C))
    w_conv = np.asarray(inputs["w_conv"], dtype=np.float32).reshape(PL, C, 9)
    w_conv_t = np.ascontiguousarray(
        w_conv.transpose(1, 2, 0).reshape(C, 9 * PL))
    bgb = np.stack([
        np.asarray(inputs["b_conv"], np.float32).reshape(PL),
        np.asarray(inputs["gamma"], np.float32).reshape(PL),
        np.asarray(inputs["beta"], np.float32).reshape(PL),
    ], axis=1)

    nc = _get_nc()
    global LAST_RESULTS
    in_maps = [
        {
            "x": np.ascontiguousarray(x[b].reshape(C, H * W)),
            "w_off": w_off_t,
            "w_conv": w_conv_t,
            "b_conv": np.ascontiguousarray(bgb),
        }
        for b in range(B)
    ]
    res = run_bass_kernel_spmd(nc, in_maps, core_ids=list(range(NCORES)))
    LAST_RESULTS = res
    out = np.stack([res.results[b]["out"].reshape(PL, H, W) for b in range(B)])
    return out.astype(np.float32)


LAST_RESULTS = None


if __name__ == "__main__":
    rng = np.random.default_rng(0)
    ins = {
        "x": rng.normal(size=(B, C, H, W)).astype(np.float32),
        "w_off": (rng.normal(size=(2 * C, C, 3, 3)) * 0.01).astype(np.float32),
        "w_conv": (rng.normal(size=(PL, C, 3, 3)) * 0.05).astype(np.float32),
        "b_conv": (rng.normal(size=(PL,)) * 0.01).astype(np.float32),
        "gamma": np.ones((PL,), np.float32),
        "beta": np.zeros((PL,), np.float32),
    }
    out = kernel(**ins)
    print("out", out.shape, out.dtype, float(np.abs(out).max()))


# revision 31
# speedup vs baseline: 1.1993x; 1.1599x over previous
"""Trainium2 Bass kernel for nn_DeformConvNet (deformable conv block).

Per-core pipeline (batch-parallel, 1 image per core, 8 cores):
  1. conv1 (C->2C, 3x3) on PE in 16-row strips; the offset-channel
     deinterleave (quirky reshape in the reference) is folded into the weight
     layout (even channels -> par0, odd -> par1) and the PSUM eviction
     (stride-2 reads) so offy/offx land contiguous per 8-row sampling strip.
  2. Deformable bilinear sample in clamp-segment form:
       S_d(i,j) = x(i-2+d, j-2) + sum_k Dx(i-2+d, j-2+k) * cx_k(i,j)
       x_off    = sum_d ty_d(i,j) * S_d(i,j)
     with Dx the horizontal difference image, cx_k = clamp01(offx+2-k)
     (exact linear interpolation; window +-2 exact while |offset| < 2),
     ty via clamp differences (ty_d = P_{d-1} - P_d with P_k = clamp01).
     Weights run as 4x-rate tensor_scalar ops; the H/V adds
     accumulate on the PE via identity matmuls (PSUM) for a subset of the
     5 row-shifts, the rest chain on DVE/Pool.
  3. conv2 (C->PL, 3x3) on PE; bias+relu fused in the PSUM eviction with
     accum_out producing the BN sums for free; Square+accum for sumsq.
  4. BN training stats: tiny [128,2] AllReduce across the 8 cores, then
     y*a+b split across ACT/DVE.
"""

import sys
import numpy as np

for _p in ("/opt/trn_rl_repo",):
    if _p not in sys.path:
        sys.path.insert(0, _p)

import concourse.bass as bass
import concourse.bacc as bacc
import concourse.mybir as mybir
import concourse.tile as tile
from concourse.bass_utils import run_bass_kernel_spmd

F32 = mybir.dt.float32
F16 = mybir.dt.float16
I16 = mybir.dt.int16
AL = mybir.AluOpType
AF = mybir.ActivationFunctionType

B, C, H, W = 8, 128, 128, 128
PL = 128
HP, WP = H + 4, W + 4      # pad-2 image for sampling window
NCORES = 8
EPS = 1e-5
NTOT = float(B * H * W)

TR = 8                     # sampling strip rows
NT = H // TR               # 16 sampling strips
SR = 16                    # conv1 strip rows
NS = H // SR               # 8 conv1 strips

# ---- tuning knobs (env-overridable for sweeps) ----
import os as _os
PE_APPS_EARLY = tuple(int(c) for c in _os.environ.get("K_PE_EARLY", "01234"))
PE_APPS_LATE = tuple(int(c) for c in _os.environ.get("K_PE_LATE", "01234"))
POOL_FRAC = float(_os.environ.get("K_POOL_FRAC", "0.10"))
C1EV_POOL = _os.environ.get("K_C1EV", "act") == "pool"
XOEV_POOL = _os.environ.get("K_XOEV", "act") == "pool"
POOL_M = int(_os.environ.get("K_POOL_M", "0"))     # m-mults per strip on Pool
OUT_F16 = _os.environ.get("K_OUT_F16", "0") == "1"
TY_POOL = _os.environ.get("K_TY_POOL", "0") == "1"
DXL_POOL = _os.environ.get("K_DXL_POOL", "0") == "1"
HEAD_CAST = int(_os.environ.get("K_HEAD_CAST", "0"))
FUSE_M = _os.environ.get("K_FUSE_M", "0") == "1"
FUSE_V = _os.environ.get("K_FUSE_V", "0") == "1"
VADD_POOL = _os.environ.get("K_VADD_POOL", "0") == "1"
BASE_DVE = int(_os.environ.get("K_BASE_DVE", "0"))  # apps whose base-add on DVE
DXL_BUFS = int(_os.environ.get("K_DXL_BUFS", "2"))
H_BUFS = int(_os.environ.get("K_H_BUFS", "2"))
VC2_BUFS = int(_os.environ.get("K_VC2_BUFS", "2"))
M_BUFS = int(_os.environ.get("K_M_BUFS", "3"))
SD_BUFS = int(_os.environ.get("K_SD_BUFS", "2"))
OFF_BUFS = int(_os.environ.get("K_OFF_BUFS", "3"))


def _emit(tc):
    nc = tc.nc
    x_in = nc.declare_dram_parameter("x", [C, H * W], F32, isOutput=False)
    woff_in = nc.declare_dram_parameter("w_off", [C, 18 * C], F32, isOutput=False)
    wconv_in = nc.declare_dram_parameter("w_conv", [C, 9 * PL], F32, isOutput=False)
    b_in = nc.declare_dram_parameter("b_conv", [PL, 3], F32, isOutput=False)
    out_o = nc.declare_dram_parameter("out", [PL, H * W], F32, isOutput=True)

    # Bresenham-style DVE/Pool round robin for TT ops
    rr = {"acc": 0.0}

    def eng_tt():
        rr["acc"] += POOL_FRAC
        if rr["acc"] >= 1.0:
            rr["acc"] -= 1.0
            return nc.gpsimd
        return nc.vector

    with (
        tc.tile_pool(name="const", bufs=1) as const,
        tc.tile_pool(name="dram", bufs=1, space="DRAM") as dram,
        tc.tile_pool(name="offp", bufs=OFF_BUFS) as offp,
        tc.tile_pool(name="wts", bufs=2) as wts,
        tc.tile_pool(name="slab", bufs=2) as slab,
        tc.tile_pool(name="ps", bufs=1, space="PSUM") as psp,
    ):
        # ---------------- loads / constants ----------------
        x16 = const.tile([C, HP * WP], F16)
        x3 = x16[:].rearrange("p (h w) -> p h w", w=WP)
        # zero the pad ring (2 wide); interior filled by DMA
        nc.vector.memset(x3[:, 0:2, :], 0.0)
        nc.vector.memset(x3[:, HP - 2:HP, :], 0.0)
        nc.vector.memset(x3[:, 2:2 + H, 0:2], 0.0)
        nc.vector.memset(x3[:, 2:2 + H, WP - 2:WP], 0.0)
        w1all = const.tile([C, 18 * C], F16)
        nc.gpsimd.dma_start(out=w1all[:], in_=woff_in[:])
        w1 = {(par, uv): w1all[:, (par * 9 + uv) * C:(par * 9 + uv + 1) * C]
              for par in range(2) for uv in range(9)}
        w2all = const.tile([C, 9 * PL], F16)
        nc.gpsimd.dma_start(out=w2all[:], in_=wconv_in[:])
        w2 = [w2all[:, uv * PL:(uv + 1) * PL] for uv in range(9)]

        bgb = const.tile([PL, 3], F32)
        nc.sync.dma_start(out=bgb[:], in_=b_in[:])

        for c16 in range(16):
            if c16 < HEAD_CAST:
                nc.gpsimd.dma_start(
                    out=x3[:, 2 + 8 * c16: 2 + 8 * (c16 + 1), 2:2 + W],
                    in_=x_in[:, 1024 * c16: 1024 * (c16 + 1)].rearrange(
                        "p (r w) -> p r w", w=W))
                continue
            xstg = slab.tile([C, 1024], F32, name=f"xstg{c16}", tag="o32",
                             bufs=2)
            nc.sync.dma_start(out=xstg[:],
                              in_=x_in[:, 1024 * c16: 1024 * (c16 + 1)])
            nc.scalar.activation(
                out=x3[:, 2 + 8 * c16: 2 + 8 * (c16 + 1), 2:2 + W],
                in_=xstg[:].rearrange("p (r w) -> p r w", w=W), func=AF.Copy)
        bias_t = bgb[:, 0:1]
        gamma_t = bgb[:, 1:2]
        beta_t = bgb[:, 2:3]

        # identity weights (f16) for PE pass-through adds
        coli = const.tile([C, C], I16)
        nc.gpsimd.iota(coli[:], pattern=[[1, C]], base=0, channel_multiplier=0)
        rowi = const.tile([C, 1], I16)
        nc.gpsimd.iota(rowi[:], pattern=[[0, 1]], base=0, channel_multiplier=1)
        colf = const.tile([C, C], F16)
        nc.vector.tensor_scalar(out=colf[:], in0=coli[:], scalar1=1.0,
                                scalar2=0.0, op0=AL.mult, op1=AL.add)
        rowf = const.tile([C, 1], F16)
        nc.vector.tensor_scalar(out=rowf[:], in0=rowi[:], scalar1=1.0,
                                scalar2=0.0, op0=AL.mult, op1=AL.add)
        ident = const.tile([C, C], F16)
        nc.vector.tensor_tensor(out=ident[:], in0=colf[:],
                                in1=rowf[:].broadcast_to((C, C)), op=AL.is_equal)

        # conv2 input: sampled image, pad-1
        xoffp = const.tile([C, (H + 2) * (W + 2)], F16)
        xo3 = xoffp[:].rearrange("p (h w) -> p h w", w=W + 2)
        nc.vector.memset(xo3[:, 0:1, :], 0.0)
        nc.vector.memset(xo3[:, H + 1:H + 2, :], 0.0)
        nc.vector.memset(xo3[:, 1:1 + H, 0:1], 0.0)
        nc.vector.memset(xo3[:, 1:1 + H, W + 1:W + 2], 0.0)

        sum2 = const.tile([PL, 2 * NT], F32)
        ssq = const.tile([PL, NT], F32)

        # first few y strips (no free xoffp rows yet) live here
        ybuf = const.tile([PL, 3 * TR * W], F16)

        # offset strips produced by conv1 evictions
        offy_reg, offx_reg = {}, {}

        def get_off(t):
            if t not in offy_reg:
                offy_reg[t] = offp.tile([C, TR * W], F16, name=f"oy{t}", tag="oy")
                offx_reg[t] = offp.tile([C, TR * W], F16, name=f"ox{t}", tag="ox")
            return offy_reg[t], offx_reg[t]

        # ---------------- conv1 strip (16 rows) ----------------
        def conv1_half(s, half):
            for b4 in (2 * half, 2 * half + 1):
                r = 16 * s + 4 * b4      # first conv-pixel row of the block
                for par in range(2):
                    ps = psp.tile([C, 512], F32, tag="c1", bufs=2)
                    for uv in range(9):
                        du, dv = uv // 3 - 1, uv % 3 - 1
                        rhs = x3[:, 2 + r + du: 2 + r + du + 4, 2 + dv: 130 + dv]
                        nc.tensor.matmul(ps[:], lhsT=w1[(par, uv)], rhs=rhs,
                                         start=(uv == 0), stop=(uv == 8))
                    t = s + 8 * par
                    oy, ox = get_off(t)
                    if C1EV_POOL:
                        nc.gpsimd.tensor_copy(out=oy[:, 256 * b4: 256 * b4 + 256],
                                              in_=ps[:, 0:512:2])
                        nc.gpsimd.tensor_copy(out=ox[:, 256 * b4: 256 * b4 + 256],
                                              in_=ps[:, 1:512:2])
                    else:
                        nc.scalar.activation(out=oy[:, 256 * b4: 256 * b4 + 256],
                                             in_=ps[:, 0:512:2], func=AF.Copy)
                        nc.scalar.activation(out=ox[:, 256 * b4: 256 * b4 + 256],
                                             in_=ps[:, 1:512:2], func=AF.Copy)

        # ---------------- sampling ----------------
        cx_reg, nty_reg, sd_reg = {}, {}, {}

        def samp_weights_x(t):
            _, ox = get_off(t)
            ox3 = ox[:].rearrange("p (r w) -> p r w", w=W)
            for col, bound, op in ((0, 0.0, AL.max), (1, -1.0, AL.max),
                                   (W - 2, 1.0, AL.min), (W - 1, 0.0, AL.min)):
                nc.vector.tensor_scalar(out=ox3[:, :, col:col + 1],
                                        in0=ox3[:, :, col:col + 1],
                                        scalar1=bound, scalar2=0.0,
                                        op0=op, op1=AL.add)
            cxa = wts.tile([C, 4 * TR * W], F16, name=f"cx{t}", tag="cx")
            NS1 = TR * W
            for k in range(4):
                cx = cxa[:, k * NS1:(k + 1) * NS1]
                nc.vector.tensor_scalar(out=cx, in0=ox[:],
                                        scalar1=float(2 - k), scalar2=0.0,
                                        op0=AL.add, op1=AL.max)
                nc.vector.tensor_scalar(out=cx, in0=cx,
                                        scalar1=1.0, scalar2=0.0,
                                        op0=AL.min, op1=AL.add)
            cx_reg[t] = cxa

        def samp_weights_y(t):
            oy, _ = get_off(t)
            oy3 = oy[:].rearrange("p (r w) -> p r w", w=W)
            # coordinate clamps only matter at image edges
            if t == 0:
                nc.vector.tensor_scalar(out=oy3[:, 0:1, :], in0=oy3[:, 0:1, :],
                                        scalar1=0.0, scalar2=0.0,
                                        op0=AL.max, op1=AL.add)
                nc.vector.tensor_scalar(out=oy3[:, 1:2, :], in0=oy3[:, 1:2, :],
                                        scalar1=-1.0, scalar2=0.0,
                                        op0=AL.max, op1=AL.add)
            if t == NT - 1:
                nc.vector.tensor_scalar(out=oy3[:, TR - 2:TR - 1, :],
                                        in0=oy3[:, TR - 2:TR - 1, :],
                                        scalar1=1.0, scalar2=0.0,
                                        op0=AL.min, op1=AL.add)
                nc.vector.tensor_scalar(out=oy3[:, TR - 1:TR, :],
                                        in0=oy3[:, TR - 1:TR, :],
                                        scalar1=0.0, scalar2=0.0,
                                        op0=AL.min, op1=AL.add)
            # vertical tents via clamp differences, in one slab:
            #   slot k+1 <- P_k = clamp01(offy + 2 - k); ty_0 = 1 - P_0,
            #   slot d <- ty_d = P_{d-1} - P_d (d=1..3), slot 4 = P_3 = ty_4
            NS1 = TR * W
            tya = wts.tile([C, 5 * NS1], F16, name=f"ty{t}", tag="ty")
            for k in range(4):
                p = tya[:, (k + 1) * NS1:(k + 2) * NS1]
                nc.vector.tensor_scalar(out=p, in0=oy[:],
                                        scalar1=float(2 - k), scalar2=0.0,
                                        op0=AL.add, op1=AL.max)
                nc.vector.tensor_scalar(out=p, in0=p,
                                        scalar1=1.0, scalar2=0.0,
                                        op0=AL.min, op1=AL.add)
            nc.vector.tensor_scalar(out=tya[:, 0:NS1],
                                    in0=tya[:, NS1:2 * NS1],
                                    scalar1=-1.0, scalar2=1.0,
                                    op0=AL.mult, op1=AL.add)
            # ty_1..3 in one overlapping-streams op (reads stay ahead of writes)
            nc.vector.tensor_tensor(out=tya[:, NS1:4 * NS1],
                                    in0=tya[:, NS1:4 * NS1],
                                    in1=tya[:, 2 * NS1:5 * NS1], op=AL.subtract)
            nty_reg[t] = tya

        dxl_reg = {}

        def prefetch_dx(t):
            r0 = TR * t
            dxl = slab.tile([C, 12 * 131], F16, name=f"dxl{t}", tag="dxl",
                            bufs=DXL_BUFS)
            dx3 = dxl[:].rearrange("p (r w) -> p r w", w=131)
            dxeng = nc.gpsimd if DXL_POOL else eng_tt()
            dxeng.tensor_tensor(out=dx3[:, :, :],
                                in0=x3[:, r0:r0 + 12, 1:132],
                                in1=x3[:, r0:r0 + 12, 0:131], op=AL.subtract)
            dxl_reg[t] = dxl

        def samp_H(t, pe_apps):
            r0 = TR * t
            NS1 = TR * W
            cxa = cx_reg[t]
            cxv = cxa[:].rearrange("p (k r w) -> p k r w", k=4, w=W)
            dxl = dxl_reg.pop(t)
            dxt = dxl.tensor
            sda = slab.tile([C, 5 * NS1], F16, name=f"sd{t}", tag="sd",
                            bufs=SD_BUFS)
            for d in range(5):
                sd = sda[:, d * NS1:(d + 1) * NS1]
                # one fused mult: m_d[k, r, j] = Dx[r0-2+d+r, j-2+k] * cx_k(r, j)
                md = slab.tile([C, 4 * NS1], F16, name=f"m{t}_{d}", tag="m",
                               bufs=M_BUFS)
                if FUSE_M:
                    dx_slab = bass.AP(dxt, dxl.offset + 131 * d,
                                      [[dxt.shape[1], C], [1, 4], [131, TR],
                                       [1, W]])
                    eng_tt().tensor_tensor(
                        out=md[:].rearrange("p (k r w) -> p k r w", k=4, w=W),
                        in0=dx_slab, in1=cxv, op=AL.mult)
                else:
                    dx3 = dxl[:].rearrange("p (r w) -> p r w", w=131)
                    for k in range(4):
                        eng_tt().tensor_tensor(
                            out=md[:, k * NS1:(k + 1) * NS1].rearrange(
                                "p (r w) -> p r w", w=W),
                            in0=dx3[:, d:d + 8, k:k + W],
                            in1=cxv[:, k], op=AL.mult)
                base = x3[:, r0 + d: r0 + d + 8, 0:W]
                if d in pe_apps:
                    ps = psp.tile([C, 1024], F32, tag="h", bufs=H_BUFS)
                    skip_base = d < BASE_DVE
                    for h2 in range(2):
                        bh = x3[:, r0 + 4 * h2 + d: r0 + 4 * h2 + d + 4, 0:W]
                        pp = ps[:, 512 * h2: 512 * h2 + 512]
                        if not skip_base:
                            nc.tensor.matmul(pp, lhsT=ident[:], rhs=bh,
                                             start=True, stop=False)
                        for k in range(4):
                            nc.tensor.matmul(
                                pp, lhsT=ident[:],
                                rhs=md[:, k * NS1 + 512 * h2:
                                       k * NS1 + 512 * h2 + 512],
                                start=(skip_base and k == 0), stop=(k == 3))
                    if skip_base:
                        nc.scalar.activation(out=sd, in_=ps[:], func=AF.Copy)
                        eng_tt().tensor_tensor(
                            out=sd.rearrange("p (r w) -> p r w", w=W),
                            in0=base,
                            in1=sd.rearrange("p (r w) -> p r w", w=W),
                            op=AL.add)
                    else:
                        nc.scalar.activation(out=sd, in_=ps[:], func=AF.Copy)
                else:
                    sd3 = sd.rearrange("p (r w) -> p r w", w=W)
                    eng_tt().tensor_tensor(
                        out=sd3, in0=base,
                        in1=md[:, 0:NS1].rearrange("p (r w) -> p r w", w=W),
                        op=AL.add)
                    for k in range(1, 4):
                        eng_tt().tensor_tensor(
                            out=sd, in0=sd, in1=md[:, k * NS1:(k + 1) * NS1],
                            op=AL.add)
            sd_reg[t] = sda

        def samp_V(t):
            r0 = TR * t
            NS1 = TR * W
            tya = nty_reg[t]
            sda = sd_reg[t]
            vda = slab.tile([C, 5 * NS1], F16, name=f"vd{t}", tag="vd", bufs=2)
            if FUSE_V:
                eng_tt().tensor_tensor(out=vda[:], in0=tya[:], in1=sda[:],
                                       op=AL.mult)
            else:
                for d in range(5):
                    eng_tt().tensor_tensor(
                        out=vda[:, d * NS1:(d + 1) * NS1],
                        in0=tya[:, d * NS1:(d + 1) * NS1],
                        in1=sda[:, d * NS1:(d + 1) * NS1], op=AL.mult)
            if VADD_POOL:
                acc = slab.tile([C, NS1], F16, name=f"vacc{t}", tag="o32",
                                bufs=2)
                nc.gpsimd.tensor_tensor(out=acc[:], in0=vda[:, 0:NS1],
                                        in1=vda[:, NS1:2 * NS1], op=AL.add)
                nc.gpsimd.tensor_tensor(out=acc[:], in0=acc[:],
                                        in1=vda[:, 2 * NS1:3 * NS1], op=AL.add)
                nc.gpsimd.tensor_tensor(out=acc[:], in0=acc[:],
                                        in1=vda[:, 3 * NS1:4 * NS1], op=AL.add)
                nc.gpsimd.tensor_tensor(
                    out=xo3[:, 1 + r0: 1 + r0 + TR, 1:1 + W],
                    in0=acc[:].rearrange("p (r w) -> p r w", w=W),
                    in1=vda[:, 4 * NS1:5 * NS1].rearrange(
                        "p (r w) -> p r w", w=W), op=AL.add)
            else:
                for h2 in range(2):
                    pv = psp.tile([C, 512], F32, tag="vc2", bufs=VC2_BUFS)
                    for d in range(5):
                        nc.tensor.matmul(pv[:], lhsT=ident[:],
                                         rhs=vda[:, d * NS1 + 512 * h2:
                                                 d * NS1 + 512 * h2 + 512],
                                         start=(d == 0), stop=(d == 4))
                    nc.scalar.activation(
                        out=xo3[:, 1 + r0 + 4 * h2: 1 + r0 + 4 * h2 + 4,
                                1:1 + W],
                        in_=pv[:].rearrange("p (r w) -> p r w", w=W),
                        func=AF.Copy)

        # ---------------- conv2 strip (8 rows) ----------------
        y_slot = {}          # t -> AP view of the stored y strip
        xo_free: set = set()  # xoffp row-blocks whose conv2 readers are done
        ybuf_used = [0]

        def y_dst(t):
            # block b (xo3 rows 8b..8b+8) is read by conv2(b-1) and conv2(b)
            for b in sorted(xo_free):
                xo_free.discard(b)
                return xo3[:, TR * b: TR * b + TR, 1:1 + W]
            i = ybuf_used[0]
            ybuf_used[0] += 1
            assert i < 3, "ybuf overflow"
            return ybuf[:, i * TR * W:(i + 1) * TR * W].rearrange(
                "p (r w) -> p r w", w=W)

        def conv2_strip(t):
            r0 = TR * t
            for b in range(NT):
                bdeps = {u for u in (b - 1, b) if 0 <= u < NT}
                if b not in xo_free and bdeps <= (c2_emitted | {t})                         and b not in y_blocks:
                    xo_free.add(b)
                    y_blocks.add(b)
            ydst = y_dst(t)
            y_slot[t] = ydst
            for h2 in range(2):
                ps = psp.tile([C, 512], F32, tag="vc2", bufs=VC2_BUFS)
                rq = r0 + 4 * h2
                for uv in range(9):
                    du, dv = uv // 3 - 1, uv % 3 - 1
                    rhs = xo3[:, 1 + rq + du: 1 + rq + du + 4, 1 + dv: 1 + dv + W]
                    nc.tensor.matmul(ps[:], lhsT=w2[uv], rhs=rhs,
                                     start=(uv == 0), stop=(uv == 8))
                nc.scalar.activation(out=ydst[:, 4 * h2: 4 * h2 + 4, :],
                                     in_=ps[:].rearrange("p (r w) -> p r w", w=W),
                                     func=AF.Relu, bias=bias_t, scale=1.0,
                                     accum_out=sum2[:, 2 * t + h2: 2 * t + h2 + 1])
            sq = slab.tile([PL, TR * W], F16, name=f"sq{t}", tag="o32", bufs=2)
            nc.scalar.activation(out=sq[:].rearrange("p (r w) -> p r w", w=W),
                                 in_=ydst, func=AF.Square,
                                 accum_out=ssq[:, t:t + 1])

        # ---------------- schedule ----------------
        order = [t for s in range(NS) for t in (s + 8, s)]
        y_blocks: set = set()
        deps = {t: {u for u in (t - 1, t, t + 1) if 0 <= u < NT} for t in range(NT)}
        v_done: set = set()
        c2_emitted: set = set()

        def emit_ready_conv2():
            for tt2 in range(NT):
                if tt2 not in c2_emitted and deps[tt2] <= v_done:
                    conv2_strip(tt2)
                    c2_emitted.add(tt2)

        for s0 in (0, 1):
            conv1_half(s0, 0)
            conv1_half(s0, 1)
        prefetch_dx(order[0])
        prefetch_dx(order[1])
        prev = None
        for n, t in enumerate(order):
            emit_ready_conv2()
            if prev is not None:
                samp_V(prev)
                v_done.add(prev)
            s_next = n // 2 + 2
            if s_next < NS:
                conv1_half(s_next, n % 2)
            if n + 2 < len(order):
                prefetch_dx(order[n + 2])
            samp_weights_x(t)
            samp_H(t, PE_APPS_EARLY if n < 8 else PE_APPS_LATE)
            samp_weights_y(t)
            prev = t
        samp_V(prev)
        v_done.add(prev)
        emit_ready_conv2()

        # ---------------- stats + collective + normalize ----------------
        st2 = const.tile([PL, 2], F32)
        nc.vector.tensor_reduce(out=st2[:, 0:1], in_=sum2[:],
                                axis=mybir.AxisListType.X, op=AL.add)
        nc.vector.tensor_reduce(out=st2[:, 1:2], in_=ssq[:],
                                axis=mybir.AxisListType.X, op=AL.add)
        cc_in = dram.tile([PL, 2], F32)
        cc_out = dram.tile([PL, 2], F32)
        nc.gpsimd.dma_start(out=cc_in[:], in_=st2[:])
        nc.gpsimd.collective_compute(
            "AllReduce", AL.add,
            replica_groups=[list(range(NCORES))],
            ins=[cc_in.opt()], outs=[cc_out.opt()],
        )
        stg = const.tile([PL, 2], F32)
        nc.gpsimd.dma_start(out=stg[:], in_=cc_out[:])

        mean = const.tile([PL, 1], F32)
        nc.vector.tensor_scalar(out=mean[:], in0=stg[:, 0:1], scalar1=1.0 / NTOT,
                                scalar2=0.0, op0=AL.mult, op1=AL.add)
        ex2 = const.tile([PL, 1], F32)
        nc.vector.tensor_scalar(out=ex2[:], in0=stg[:, 1:2], scalar1=1.0 / NTOT,
                                scalar2=0.0, op0=AL.mult, op1=AL.add)
        var = const.tile([PL, 1], F32)
        nc.vector.tensor_tensor(out=var[:], in0=mean[:], in1=mean[:], op=AL.mult)
        nc.vector.tensor_tensor(out=var[:], in0=ex2[:], in1=var[:], op=AL.subtract)
        epst = const.tile([PL, 1], F32)
        nc.gpsimd.memset(epst[:], EPS)
        stdv = const.tile([PL, 1], F32)
        nc.scalar.activation(out=stdv[:], in_=var[:], func=AF.Sqrt, bias=epst[:])
        rstd = const.tile([PL, 1], F32)
        nc.vector.reciprocal(rstd[:], stdv[:])
        avec = const.tile([PL, 1], F32)
        nc.vector.tensor_tensor(out=avec[:], in0=gamma_t, in1=rstd[:], op=AL.mult)
        bvec = const.tile([PL, 1], F32)
        nc.vector.tensor_tensor(out=bvec[:], in0=avec[:], in1=mean[:], op=AL.mult)
        nc.vector.tensor_tensor(out=bvec[:], in0=beta_t, in1=bvec[:],
                                op=AL.subtract)

        for t in range(NT):
            r0 = TR * t
            ysrc = y_slot[t]
            if OUT_F16:
                ot = slab.tile([PL, TR * W], F16, name=f"o16{t}", tag="o32", bufs=2)
            else:
                ot = slab.tile([PL, TR * W], F32, name=f"o32{t}", tag="o32",
                               bufs=2)
            o3v = ot[:].rearrange("p (r w) -> p r w", w=W)
            if t % 2 == 0:
                nc.scalar.activation(out=o3v, in_=ysrc, func=AF.Identity,
                                     bias=bvec[:], scale=avec[:])
            else:
                nc.vector.tensor_scalar(out=o3v, in0=ysrc, scalar1=avec[:],
                                        scalar2=bvec[:], op0=AL.mult, op1=AL.add)
            if OUT_F16:
                nc.gpsimd.dma_start(out=out_o[:, r0 * W:(r0 + TR) * W],
                                    in_=ot[:])
            else:
                nc.sync.dma_start(out=out_o[:, r0 * W:(r0 + TR) * W],
                                  in_=ot[:])


_NC_CACHE = None


def _get_nc():
    global _NC_CACHE
    if _NC_CACHE is None:
        nc = bacc.Bacc("TRN2", target_bir_lowering=False, debug=False,
                       num_devices=NCORES)
        with tile.TileContext(nc) as tc:
            _emit(tc)
        nc.compile()
        _NC_CACHE = nc
    return _NC_CACHE


def kernel(**inputs):
    x = np.ascontiguousarray(np.asarray(inputs["x"], dtype=np.float32))
    w_off = np.asarray(inputs["w_off"], dtype=np.float32).reshape(C, 2, C, 9)
    w_off_t = np.ascontiguousarray(
        w_off.transpose(2, 1, 3, 0).reshape(C, 18 * C))
    w_conv = np.asarray(inputs["w_conv"], dtype=np.float32).reshape(PL, C, 9)
    w_conv_t = np.ascontiguousarray(
        w_conv.transpose(1, 2, 0).reshape(C, 9 * PL))
    bgb = np.stack([
        np.asarray(inputs["b_conv"], np.float32).reshape(PL),
        np.asarray(inputs["gamma"], np.float32).reshape(PL),
        np.asarray(inputs["beta"], np.float32).reshape(PL),
    ], axis=1)

    nc = _get_nc()
    global LAST_RESULTS
    in_maps = [
        {
            "x": np.ascontiguousarray(x[b].reshape(C, H * W)),
            "w_off": w_off_t,
            "w_conv": w_conv_t,
            "b_conv": np.ascontiguousarray(bgb),
        }
        for b in range(B)
    ]
    res = run_bass_kernel_spmd(nc, in_maps, core_ids=list(range(NCORES)))
    LAST_RESULTS = res
    out = np.stack([res.results[b]["out"].reshape(PL, H, W) for b in range(B)])
    return out.astype(np.float32)


LAST_RESULTS = None


if __name__ == "__main__":
    rng = np.random.default_rng(0)
    ins = {
        "x": rng.normal(size=(B, C, H, W)).astype(np.float32),
        "w_off": (rng.normal(size=(2 * C, C, 3, 3)) * 0.01).astype(np.float32),
        "w_conv": (rng.normal(size=(PL, C, 3, 3)) * 0.05).astype(np.float32),
        "b_conv": (rng.normal(size=(PL,)) * 0.01).astype(np.float32),
        "gamma": np.ones((PL,), np.float32),
        "beta": np.zeros((PL,), np.float32),
    }
    out = kernel(**ins)
    print("out", out.shape, out.dtype, float(np.abs(out).max()))


# revision 34
# speedup vs baseline: 1.2296x; 1.0253x over previous
"""Trainium2 Bass kernel for nn_DeformConvNet (deformable conv block).

Per-core pipeline (batch-parallel, 1 image per core, 8 cores):
  1. conv1 (C->2C, 3x3) on PE in 16-row strips; the offset-channel
     deinterleave (quirky reshape in the reference) is folded into the weight
     layout (even channels -> par0, odd -> par1) and the PSUM eviction
     (stride-2 reads) so offy/offx land contiguous per 8-row sampling strip.
  2. Deformable bilinear sample in clamp-segment form:
       S_d(i,j) = x(i-2+d, j-2) + sum_k Dx(i-2+d, j-2+k) * cx_k(i,j)
       x_off    = sum_d ty_d(i,j) * S_d(i,j)
     with Dx the horizontal difference image, cx_k = clamp01(offx+2-k)
     (exact linear interpolation; window +-2 exact while |offset| < 2),
     ty via clamp differences (ty_d = P_{d-1} - P_d with P_k = clamp01).
     Weights run as 4x-rate tensor_scalar ops; the H/V adds
     accumulate on the PE via identity matmuls (PSUM) for a subset of the
     5 row-shifts, the rest chain on DVE/Pool.
  3. conv2 (C->PL, 3x3) on PE; bias+relu fused in the PSUM eviction with
     accum_out producing the BN sums for free; Square+accum for sumsq.
  4. BN training stats: tiny [128,2] AllReduce across the 8 cores, then
     y*a+b split across ACT/DVE.
"""

import sys
import numpy as np

for _p in ("/opt/trn_rl_repo",):
    if _p not in sys.path:
        sys.path.insert(0, _p)

import concourse.bass as bass
import concourse.bacc as bacc
import concourse.mybir as mybir
import concourse.tile as tile
from concourse.bass_utils import run_bass_kernel_spmd

F32 = mybir.dt.float32
F16 = mybir.dt.float16
I16 = mybir.dt.int16
AL = mybir.AluOpType
AF = mybir.ActivationFunctionType

B, C, H, W = 8, 128, 128, 128
PL = 128
HP, WP = H + 4, W + 4      # pad-2 image for sampling window
NCORES = 8
EPS = 1e-5
NTOT = float(B * H * W)

TR = 8                     # sampling strip rows
NT = H // TR               # 16 sampling strips
SR = 16                    # conv1 strip rows
NS = H // SR               # 8 conv1 strips

# ---- tuning knobs (env-overridable for sweeps) ----
import os as _os
PE_APPS_EARLY = tuple(int(c) for c in _os.environ.get("K_PE_EARLY", "01234"))
PE_APPS_LATE = tuple(int(c) for c in _os.environ.get("K_PE_LATE", "01234"))
POOL_FRAC = float(_os.environ.get("K_POOL_FRAC", "0.10"))
C1EV_POOL = _os.environ.get("K_C1EV", "act") == "pool"
XOEV_POOL = _os.environ.get("K_XOEV", "act") == "pool"
POOL_M = int(_os.environ.get("K_POOL_M", "0"))     # m-mults per strip on Pool
OUT_F16 = _os.environ.get("K_OUT_F16", "0") == "1"
TY_POOL = _os.environ.get("K_TY_POOL", "0") == "1"
DXL_POOL = _os.environ.get("K_DXL_POOL", "0") == "1"
HEAD_CAST = int(_os.environ.get("K_HEAD_CAST", "0"))
FUSE_M = _os.environ.get("K_FUSE_M", "0") == "1"
FUSE_V = _os.environ.get("K_FUSE_V", "0") == "1"
VADD_POOL = _os.environ.get("K_VADD_POOL", "0") == "1"
BASE_DVE = int(_os.environ.get("K_BASE_DVE", "0"))  # apps whose base-add on DVE
DXL_BUFS = int(_os.environ.get("K_DXL_BUFS", "2"))
H_BUFS = int(_os.environ.get("K_H_BUFS", "2"))
VC2_BUFS = int(_os.environ.get("K_VC2_BUFS", "2"))
M_BUFS = int(_os.environ.get("K_M_BUFS", "3"))
SD_BUFS = int(_os.environ.get("K_SD_BUFS", "2"))
OFF_BUFS = int(_os.environ.get("K_OFF_BUFS", "3"))


def _emit(tc):
    nc = tc.nc
    x_in = nc.declare_dram_parameter("x", [C, H * W], F32, isOutput=False)
    woff_in = nc.declare_dram_parameter("w_off", [C, 18 * C], F32, isOutput=False)
    wconv_in = nc.declare_dram_parameter("w_conv", [C, 9 * PL], F32, isOutput=False)
    b_in = nc.declare_dram_parameter("b_conv", [PL, 3], F32, isOutput=False)
    out_o = nc.declare_dram_parameter("out", [PL, H * W], F32, isOutput=True)

    # Bresenham-style DVE/Pool round robin for TT ops
    rr = {"acc": 0.0}

    def eng_tt():
        rr["acc"] += POOL_FRAC
        if rr["acc"] >= 1.0:
            rr["acc"] -= 1.0
            return nc.gpsimd
        return nc.vector

    with (
        tc.tile_pool(name="const", bufs=1) as const,
        tc.tile_pool(name="dram", bufs=1, space="DRAM") as dram,
        tc.tile_pool(name="offp", bufs=OFF_BUFS) as offp,
        tc.tile_pool(name="wts", bufs=2) as wts,
        tc.tile_pool(name="slab", bufs=2) as slab,
        tc.tile_pool(name="ps", bufs=1, space="PSUM") as psp,
    ):
        # ---------------- loads / constants ----------------
        x16 = const.tile([C, HP * WP], F16)
        x3 = x16[:].rearrange("p (h w) -> p h w", w=WP)
        # zero the pad ring (2 wide); interior filled by DMA
        nc.vector.memset(x3[:, 0:2, :], 0.0)
        nc.vector.memset(x3[:, HP - 2:HP, :], 0.0)
        nc.vector.memset(x3[:, 2:2 + H, 0:2], 0.0)
        nc.vector.memset(x3[:, 2:2 + H, WP - 2:WP], 0.0)
        w1all = const.tile([C, 18 * C], F16)
        nc.gpsimd.dma_start(out=w1all[:], in_=woff_in[:])
        w1 = {(par, uv): w1all[:, (par * 9 + uv) * C:(par * 9 + uv + 1) * C]
              for par in range(2) for uv in range(9)}
        w2all = const.tile([C, 9 * PL], F16)
        nc.gpsimd.dma_start(out=w2all[:], in_=wconv_in[:])
        w2 = [w2all[:, uv * PL:(uv + 1) * PL] for uv in range(9)]

        bgb = const.tile([PL, 3], F32)
        nc.sync.dma_start(out=bgb[:], in_=b_in[:])

        for c16 in range(16):
            if c16 < HEAD_CAST:
                nc.gpsimd.dma_start(
                    out=x3[:, 2 + 8 * c16: 2 + 8 * (c16 + 1), 2:2 + W],
                    in_=x_in[:, 1024 * c16: 1024 * (c16 + 1)].rearrange(
                        "p (r w) -> p r w", w=W))
                continue
            xstg = slab.tile([C, 1024], F32, name=f"xstg{c16}", tag="o32",
                             bufs=2)
            nc.sync.dma_start(out=xstg[:],
                              in_=x_in[:, 1024 * c16: 1024 * (c16 + 1)])
            nc.scalar.activation(
                out=x3[:, 2 + 8 * c16: 2 + 8 * (c16 + 1), 2:2 + W],
                in_=xstg[:].rearrange("p (r w) -> p r w", w=W), func=AF.Copy)
        bias_t = bgb[:, 0:1]
        gamma_t = bgb[:, 1:2]
        beta_t = bgb[:, 2:3]

        # identity weights (f16) for PE pass-through adds
        coli = const.tile([C, C], I16)
        nc.gpsimd.iota(coli[:], pattern=[[1, C]], base=0, channel_multiplier=0)
        rowi = const.tile([C, 1], I16)
        nc.gpsimd.iota(rowi[:], pattern=[[0, 1]], base=0, channel_multiplier=1)
        colf = const.tile([C, C], F16)
        nc.vector.tensor_scalar(out=colf[:], in0=coli[:], scalar1=1.0,
                                scalar2=0.0, op0=AL.mult, op1=AL.add)
        rowf = const.tile([C, 1], F16)
        nc.vector.tensor_scalar(out=rowf[:], in0=rowi[:], scalar1=1.0,
                                scalar2=0.0, op0=AL.mult, op1=AL.add)
        ident = const.tile([C, C], F16)
        nc.vector.tensor_tensor(out=ident[:], in0=colf[:],
                                in1=rowf[:].broadcast_to((C, C)), op=AL.is_equal)

        # conv2 input: sampled image, pad-1
        xoffp = const.tile([C, (H + 2) * (W + 2)], F16)
        xo3 = xoffp[:].rearrange("p (h w) -> p h w", w=W + 2)
        nc.vector.memset(xo3[:, 0:1, :], 0.0)
        nc.vector.memset(xo3[:, H + 1:H + 2, :], 0.0)
        nc.vector.memset(xo3[:, 1:1 + H, 0:1], 0.0)
        nc.vector.memset(xo3[:, 1:1 + H, W + 1:W + 2], 0.0)

        sum2 = const.tile([PL, 2 * NT], F32)
        ssq = const.tile([PL, NT], F32)

        # first few y strips (no free xoffp rows yet) live here
        ybuf = const.tile([PL, 3 * TR * W], F16)

        # offset strips produced by conv1 evictions
        offy_reg, offx_reg = {}, {}

        def get_off(t):
            if t not in offy_reg:
                offy_reg[t] = offp.tile([C, TR * W], F16, name=f"oy{t}", tag="oy")
                offx_reg[t] = offp.tile([C, TR * W], F16, name=f"ox{t}", tag="ox")
            return offy_reg[t], offx_reg[t]

        # ---------------- conv1 strip (16 rows) ----------------
        def conv1_half(s, half):
            for b4 in (2 * half, 2 * half + 1):
                r = 16 * s + 4 * b4      # first conv-pixel row of the block
                for par in range(2):
                    ps = psp.tile([C, 512], F32, tag="c1", bufs=2)
                    for uv in range(9):
                        du, dv = uv // 3 - 1, uv % 3 - 1
                        rhs = x3[:, 2 + r + du: 2 + r + du + 4, 2 + dv: 130 + dv]
                        nc.tensor.matmul(ps[:], lhsT=w1[(par, uv)], rhs=rhs,
                                         start=(uv == 0), stop=(uv == 8))
                    t = s + 8 * par
                    oy, ox = get_off(t)
                    if C1EV_POOL:
                        nc.gpsimd.tensor_copy(out=oy[:, 256 * b4: 256 * b4 + 256],
                                              in_=ps[:, 0:512:2])
                        nc.gpsimd.tensor_copy(out=ox[:, 256 * b4: 256 * b4 + 256],
                                              in_=ps[:, 1:512:2])
                    else:
                        nc.scalar.activation(out=oy[:, 256 * b4: 256 * b4 + 256],
                                             in_=ps[:, 0:512:2], func=AF.Copy)
                        nc.scalar.activation(out=ox[:, 256 * b4: 256 * b4 + 256],
                                             in_=ps[:, 1:512:2], func=AF.Copy)

        # ---------------- sampling ----------------
        cx_reg, nty_reg, sd_reg = {}, {}, {}

        def samp_weights_x(t):
            _, ox = get_off(t)
            ox3 = ox[:].rearrange("p (r w) -> p r w", w=W)
            for col, bound, op in ((0, 0.0, AL.max), (1, -1.0, AL.max),
                                   (W - 2, 1.0, AL.min), (W - 1, 0.0, AL.min)):
                nc.vector.tensor_scalar(out=ox3[:, :, col:col + 1],
                                        in0=ox3[:, :, col:col + 1],
                                        scalar1=bound, scalar2=0.0,
                                        op0=op, op1=AL.add)
            cxa = wts.tile([C, 4 * TR * W], F16, name=f"cx{t}", tag="cx")
            NS1 = TR * W
            for k in range(4):
                cx = cxa[:, k * NS1:(k + 1) * NS1]
                nc.vector.tensor_scalar(out=cx, in0=ox[:],
                                        scalar1=float(2 - k), scalar2=0.0,
                                        op0=AL.add, op1=AL.max)
                nc.vector.tensor_scalar(out=cx, in0=cx,
                                        scalar1=1.0, scalar2=0.0,
                                        op0=AL.min, op1=AL.add)
            cx_reg[t] = cxa

        def samp_weights_y(t):
            oy, _ = get_off(t)
            oy3 = oy[:].rearrange("p (r w) -> p r w", w=W)
            # coordinate clamps only matter at image edges
            if t == 0:
                nc.vector.tensor_scalar(out=oy3[:, 0:1, :], in0=oy3[:, 0:1, :],
                                        scalar1=0.0, scalar2=0.0,
                                        op0=AL.max, op1=AL.add)
                nc.vector.tensor_scalar(out=oy3[:, 1:2, :], in0=oy3[:, 1:2, :],
                                        scalar1=-1.0, scalar2=0.0,
                                        op0=AL.max, op1=AL.add)
            if t == NT - 1:
                nc.vector.tensor_scalar(out=oy3[:, TR - 2:TR - 1, :],
                                        in0=oy3[:, TR - 2:TR - 1, :],
                                        scalar1=1.0, scalar2=0.0,
                                        op0=AL.min, op1=AL.add)
                nc.vector.tensor_scalar(out=oy3[:, TR - 1:TR, :],
                                        in0=oy3[:, TR - 1:TR, :],
                                        scalar1=0.0, scalar2=0.0,
                                        op0=AL.min, op1=AL.add)
            # vertical tents via clamp differences, in one slab:
            #   slot k+1 <- P_k = clamp01(offy + 2 - k); ty_0 = 1 - P_0,
            #   slot d <- ty_d = P_{d-1} - P_d (d=1..3), slot 4 = P_3 = ty_4
            NS1 = TR * W
            tya = wts.tile([C, 5 * NS1], F16, name=f"ty{t}", tag="ty")
            for k in range(4):
                p = tya[:, (k + 1) * NS1:(k + 2) * NS1]
                nc.vector.tensor_scalar(out=p, in0=oy[:],
                                        scalar1=float(2 - k), scalar2=0.0,
                                        op0=AL.add, op1=AL.max)
                nc.vector.tensor_scalar(out=p, in0=p,
                                        scalar1=1.0, scalar2=0.0,
                                        op0=AL.min, op1=AL.add)
            nc.vector.tensor_scalar(out=tya[:, 0:NS1],
                                    in0=tya[:, NS1:2 * NS1],
                                    scalar1=-1.0, scalar2=1.0,
                                    op0=AL.mult, op1=AL.add)
            # ty_1..3 in one overlapping-streams op (reads stay ahead of writes)
            nc.vector.tensor_tensor(out=tya[:, NS1:4 * NS1],
                                    in0=tya[:, NS1:4 * NS1],
                                    in1=tya[:, 2 * NS1:5 * NS1], op=AL.subtract)
            nty_reg[t] = tya

        dxl_reg = {}

        def prefetch_dx(t):
            r0 = TR * t
            dxl = slab.tile([C, 12 * 131], F16, name=f"dxl{t}", tag="dxl",
                            bufs=DXL_BUFS)
            dx3 = dxl[:].rearrange("p (r w) -> p r w", w=131)
            dxeng = nc.gpsimd if DXL_POOL else eng_tt()
            dxeng.tensor_tensor(out=dx3[:, :, :],
                                in0=x3[:, r0:r0 + 12, 1:132],
                                in1=x3[:, r0:r0 + 12, 0:131], op=AL.subtract)
            dxl_reg[t] = dxl

        def samp_H(t, pe_apps):
            r0 = TR * t
            NS1 = TR * W
            cxa = cx_reg[t]
            cxv = cxa[:].rearrange("p (k r w) -> p k r w", k=4, w=W)
            dxl = dxl_reg.pop(t)
            dxt = dxl.tensor
            sda = slab.tile([C, 5 * NS1], F16, name=f"sd{t}", tag="sd",
                            bufs=SD_BUFS)
            for d in range(5):
                sd = sda[:, d * NS1:(d + 1) * NS1]
                # one fused mult: m_d[k, r, j] = Dx[r0-2+d+r, j-2+k] * cx_k(r, j)
                md = slab.tile([C, 4 * NS1], F16, name=f"m{t}_{d}", tag="m",
                               bufs=M_BUFS)
                if FUSE_M:
                    dx_slab = bass.AP(dxt, dxl.offset + 131 * d,
                                      [[dxt.shape[1], C], [1, 4], [131, TR],
                                       [1, W]])
                    eng_tt().tensor_tensor(
                        out=md[:].rearrange("p (k r w) -> p k r w", k=4, w=W),
                        in0=dx_slab, in1=cxv, op=AL.mult)
                else:
                    dx3 = dxl[:].rearrange("p (r w) -> p r w", w=131)
                    for k in range(4):
                        eng_tt().tensor_tensor(
                            out=md[:, k * NS1:(k + 1) * NS1].rearrange(
                                "p (r w) -> p r w", w=W),
                            in0=dx3[:, d:d + 8, k:k + W],
                            in1=cxv[:, k], op=AL.mult)
                base = x3[:, r0 + d: r0 + d + 8, 0:W]
                if d in pe_apps:
                    ps = psp.tile([C, 1024], F32, tag="h", bufs=H_BUFS)
                    skip_base = d < BASE_DVE
                    for h2 in range(2):
                        bh = x3[:, r0 + 4 * h2 + d: r0 + 4 * h2 + d + 4, 0:W]
                        pp = ps[:, 512 * h2: 512 * h2 + 512]
                        if not skip_base:
                            nc.tensor.matmul(pp, lhsT=ident[:], rhs=bh,
                                             start=True, stop=False)
                        for k in range(4):
                            nc.tensor.matmul(
                                pp, lhsT=ident[:],
                                rhs=md[:, k * NS1 + 512 * h2:
                                       k * NS1 + 512 * h2 + 512],
                                start=(skip_base and k == 0), stop=(k == 3))
                    if skip_base:
                        nc.scalar.activation(out=sd, in_=ps[:], func=AF.Copy)
                        eng_tt().tensor_tensor(
                            out=sd.rearrange("p (r w) -> p r w", w=W),
                            in0=base,
                            in1=sd.rearrange("p (r w) -> p r w", w=W),
                            op=AL.add)
                    else:
                        nc.scalar.activation(out=sd, in_=ps[:], func=AF.Copy)
                else:
                    sd3 = sd.rearrange("p (r w) -> p r w", w=W)
                    eng_tt().tensor_tensor(
                        out=sd3, in0=base,
                        in1=md[:, 0:NS1].rearrange("p (r w) -> p r w", w=W),
                        op=AL.add)
                    for k in range(1, 4):
                        eng_tt().tensor_tensor(
                            out=sd, in0=sd, in1=md[:, k * NS1:(k + 1) * NS1],
                            op=AL.add)
            sd_reg[t] = sda

        def samp_V(t):
            r0 = TR * t
            NS1 = TR * W
            tya = nty_reg[t]
            sda = sd_reg[t]
            vda = slab.tile([C, 5 * NS1], F16, name=f"vd{t}", tag="vd", bufs=2)
            if FUSE_V:
                eng_tt().tensor_tensor(out=vda[:], in0=tya[:], in1=sda[:],
                                       op=AL.mult)
            else:
                for d in range(5):
                    eng_tt().tensor_tensor(
                        out=vda[:, d * NS1:(d + 1) * NS1],
                        in0=tya[:, d * NS1:(d + 1) * NS1],
                        in1=sda[:, d * NS1:(d + 1) * NS1], op=AL.mult)
            if VADD_POOL:
                acc = slab.tile([C, NS1], F16, name=f"vacc{t}", tag="o32",
                                bufs=2)
                nc.gpsimd.tensor_tensor(out=acc[:], in0=vda[:, 0:NS1],
                                        in1=vda[:, NS1:2 * NS1], op=AL.add)
                nc.gpsimd.tensor_tensor(out=acc[:], in0=acc[:],
                                        in1=vda[:, 2 * NS1:3 * NS1], op=AL.add)
                nc.gpsimd.tensor_tensor(out=acc[:], in0=acc[:],
                                        in1=vda[:, 3 * NS1:4 * NS1], op=AL.add)
                nc.gpsimd.tensor_tensor(
                    out=xo3[:, 1 + r0: 1 + r0 + TR, 1:1 + W],
                    in0=acc[:].rearrange("p (r w) -> p r w", w=W),
                    in1=vda[:, 4 * NS1:5 * NS1].rearrange(
                        "p (r w) -> p r w", w=W), op=AL.add)
            else:
                for h2 in range(2):
                    pv = psp.tile([C, 512], F32, tag="vc2", bufs=VC2_BUFS)
                    for d in range(5):
                        nc.tensor.matmul(pv[:], lhsT=ident[:],
                                         rhs=vda[:, d * NS1 + 512 * h2:
                                                 d * NS1 + 512 * h2 + 512],
                                         start=(d == 0), stop=(d == 4))
                    nc.scalar.activation(
                        out=xo3[:, 1 + r0 + 4 * h2: 1 + r0 + 4 * h2 + 4,
                                1:1 + W],
                        in_=pv[:].rearrange("p (r w) -> p r w", w=W),
                        func=AF.Copy)

        # ---------------- conv2 strip (8 rows) ----------------
        y_slot = {}          # t -> AP view of the stored y strip
        xo_free: set = set()  # xoffp row-blocks whose conv2 readers are done
        ybuf_used = [0]

        def y_dst(t):
            # block b (xo3 rows 8b..8b+8) is read by conv2(b-1) and conv2(b)
            for b in sorted(xo_free):
                xo_free.discard(b)
                return xo3[:, TR * b: TR * b + TR, 1:1 + W]
            i = ybuf_used[0]
            ybuf_used[0] += 1
            assert i < 3, "ybuf overflow"
            return ybuf[:, i * TR * W:(i + 1) * TR * W].rearrange(
                "p (r w) -> p r w", w=W)

        def conv2_strip(t):
            r0 = TR * t
            for b in range(NT):
                bdeps = {u for u in (b - 1, b) if 0 <= u < NT}
                if b not in xo_free and bdeps <= (c2_emitted | {t})                         and b not in y_blocks:
                    xo_free.add(b)
                    y_blocks.add(b)
            ydst = y_dst(t)
            y_slot[t] = ydst
            for h2 in range(2):
                ps = psp.tile([C, 512], F32, tag="vc2", bufs=VC2_BUFS)
                rq = r0 + 4 * h2
                for uv in range(9):
                    du, dv = uv // 3 - 1, uv % 3 - 1
                    rhs = xo3[:, 1 + rq + du: 1 + rq + du + 4, 1 + dv: 1 + dv + W]
                    nc.tensor.matmul(ps[:], lhsT=w2[uv], rhs=rhs,
                                     start=(uv == 0), stop=(uv == 8))
                nc.scalar.activation(out=ydst[:, 4 * h2: 4 * h2 + 4, :],
                                     in_=ps[:].rearrange("p (r w) -> p r w", w=W),
                                     func=AF.Relu, bias=bias_t, scale=1.0,
                                     accum_out=sum2[:, 2 * t + h2: 2 * t + h2 + 1])
            sq = slab.tile([PL, TR * W], F16, name=f"sq{t}", tag="o32", bufs=2)
            nc.scalar.activation(out=sq[:].rearrange("p (r w) -> p r w", w=W),
                                 in_=ydst, func=AF.Square,
                                 accum_out=ssq[:, t:t + 1])

        # ---------------- schedule ----------------
        order = [t for s in range(NS) for t in (s + 8, s)]
        y_blocks: set = set()
        deps = {t: {u for u in (t - 1, t, t + 1) if 0 <= u < NT} for t in range(NT)}
        v_done: set = set()
        c2_emitted: set = set()

        def emit_ready_conv2():
            for tt2 in range(NT):
                if tt2 not in c2_emitted and deps[tt2] <= v_done:
                    conv2_strip(tt2)
                    c2_emitted.add(tt2)

        for s0 in (0, 1):
            conv1_half(s0, 0)
            conv1_half(s0, 1)
        prefetch_dx(order[0])
        prefetch_dx(order[1])
        prev = None
        for n, t in enumerate(order):
            emit_ready_conv2()
            if prev is not None:
                samp_V(prev)
                v_done.add(prev)
            s_next = n // 2 + 2
            if s_next < NS:
                conv1_half(s_next, n % 2)
            if n + 2 < len(order):
                prefetch_dx(order[n + 2])
            samp_weights_x(t)
            samp_H(t, PE_APPS_EARLY if n < 8 else PE_APPS_LATE)
            samp_weights_y(t)
            prev = t
        samp_V(prev)
        v_done.add(prev)
        emit_ready_conv2()

        # ---------------- stats + collective + normalize ----------------
        st2 = const.tile([PL, 2], F32)
        nc.vector.tensor_reduce(out=st2[:, 0:1], in_=sum2[:],
                                axis=mybir.AxisListType.X, op=AL.add)
        nc.vector.tensor_reduce(out=st2[:, 1:2], in_=ssq[:],
                                axis=mybir.AxisListType.X, op=AL.add)
        cc_in = dram.tile([PL, 2], F32)
        cc_out = dram.tile([PL, 2 * NCORES], F32)
        nc.gpsimd.dma_start(out=cc_in[:], in_=st2[:])
        # AllGather + local sum: same result as AllReduce but ~13us cheaper
        nc.gpsimd.collective_compute(
            "AllGather", AL.bypass,
            replica_groups=[list(range(NCORES))],
            ins=[cc_in.opt()], outs=[cc_out.opt()],
        )
        stg = const.tile([PL, 2 * NCORES], F32)
        # unscramble the flat core-major gather: stg[q, 2c+j] = flat[256c+2q+j]
        cc_view = bass.AP(cc_out.tensor, 0,
                          [[2, PL], [2 * PL, NCORES], [1, 2]])
        nc.gpsimd.dma_start(
            out=stg[:].rearrange("p (c two) -> p c two", two=2), in_=cc_view)
        stg3 = stg[:].rearrange("p (c two) -> p two c", two=2)

        sumv = const.tile([PL, 1], F32)
        nc.vector.tensor_reduce(out=sumv[:], in_=stg3[:, 0],
                                axis=mybir.AxisListType.X, op=AL.add)
        ssqv = const.tile([PL, 1], F32)
        nc.vector.tensor_reduce(out=ssqv[:], in_=stg3[:, 1],
                                axis=mybir.AxisListType.X, op=AL.add)

        mean = const.tile([PL, 1], F32)
        nc.vector.tensor_scalar(out=mean[:], in0=sumv[:], scalar1=1.0 / NTOT,
                                scalar2=0.0, op0=AL.mult, op1=AL.add)
        ex2 = const.tile([PL, 1], F32)
        nc.vector.tensor_scalar(out=ex2[:], in0=ssqv[:], scalar1=1.0 / NTOT,
                                scalar2=0.0, op0=AL.mult, op1=AL.add)
        var = const.tile([PL, 1], F32)
        nc.vector.tensor_tensor(out=var[:], in0=mean[:], in1=mean[:], op=AL.mult)
        nc.vector.tensor_tensor(out=var[:], in0=ex2[:], in1=var[:], op=AL.subtract)
        epst = const.tile([PL, 1], F32)
        nc.gpsimd.memset(epst[:], EPS)
        stdv = const.tile([PL, 1], F32)
        nc.scalar.activation(out=stdv[:], in_=var[:], func=AF.Sqrt, bias=epst[:])
        rstd = const.tile([PL, 1], F32)
        nc.vector.reciprocal(rstd[:], stdv[:])
        avec = const.tile([PL, 1], F32)
        nc.vector.tensor_tensor(out=avec[:], in0=gamma_t, in1=rstd[:], op=AL.mult)
        bvec = const.tile([PL, 1], F32)
        nc.vector.tensor_tensor(out=bvec[:], in0=avec[:], in1=mean[:], op=AL.mult)
        nc.vector.tensor_tensor(out=bvec[:], in0=beta_t, in1=bvec[:],
                                op=AL.subtract)

        for t in range(NT):
            r0 = TR * t
            ysrc = y_slot[t]
            if OUT_F16:
                ot = slab.tile([PL, TR * W], F16, name=f"o16{t}", tag="o32", bufs=2)
            else:
                ot = slab.tile([PL, TR * W], F32, name=f"o32{t}", tag="o32",
                               bufs=2)
            o3v = ot[:].rearrange("p (r w) -> p r w", w=W)
            if t % 2 == 0:
                nc.scalar.activation(out=o3v, in_=ysrc, func=AF.Identity,
                                     bias=bvec[:], scale=avec[:])
            else:
                nc.vector.tensor_scalar(out=o3v, in0=ysrc, scalar1=avec[:],
                                        scalar2=bvec[:], op0=AL.mult, op1=AL.add)
            if OUT_F16:
                nc.gpsimd.dma_start(out=out_o[:, r0 * W:(r0 + TR) * W],
                                    in_=ot[:])
            else:
                nc.sync.dma_start(out=out_o[:, r0 * W:(r0 + TR) * W],
                                  in_=ot[:])


_NC_CACHE = None


def _get_nc():
    global _NC_CACHE
    if _NC_CACHE is None:
        nc = bacc.Bacc("TRN2", target_bir_lowering=False, debug=False,
                       num_devices=NCORES)
        with tile.TileContext(nc) as tc:
            _emit(tc)
        nc.compile()
        _NC_CACHE = nc
    return _NC_CACHE


def kernel(**inputs):
    x = np.ascontiguousarray(np.asarray(inputs["x"], dtype=np.float32))
    w_off = np.asarray(inputs["w_off"], dtype=np.float32).reshape(C, 2, C, 9)
    w_off_t = np.ascontiguousarray(
        w_off.transpose(2, 1, 3, 0).reshape(C, 18 * C))
    w_conv = np.asarray(inputs["w_conv"], dtype=np.float32).reshape(PL, C, 9)
    w_conv_t = np.ascontiguousarray(
        w_conv.transpose(1, 2, 0).reshape(C, 9 * PL))
    bgb = np.stack([
        np.asarray(inputs["b_conv"], np.float32).reshape(PL),
        np.asarray(inputs["gamma"], np.float32).reshape(PL),
        np.asarray(inputs["beta"], np.float32).reshape(PL),
    ], axis=1)

    nc = _get_nc()
    global LAST_RESULTS
    in_maps = [
        {
            "x": np.ascontiguousarray(x[b].reshape(C, H * W)),
            "w_off": w_off_t,
            "w_conv": w_conv_t,
            "b_conv": np.ascontiguousarray(bgb),
        }
        for b in range(B)
    ]
    res = run_bass_kernel_spmd(nc, in_maps, core_ids=list(range(NCORES)))
    LAST_RESULTS = res
    out = np.stack([res.results[b]["out"].reshape(PL, H, W) for b in range(B)])
    return out.astype(np.float32)


LAST_RESULTS = None


if __name__ == "__main__":
    rng = np.random.default_rng(0)
    ins = {
        "x": rng.normal(size=(B, C, H, W)).astype(np.float32),
        "w_off": (rng.normal(size=(2 * C, C, 3, 3)) * 0.01).astype(np.float32),
        "w_conv": (rng.normal(size=(PL, C, 3, 3)) * 0.05).astype(np.float32),
        "b_conv": (rng.normal(size=(PL,)) * 0.01).astype(np.float32),
        "gamma": np.ones((PL,), np.float32),
        "beta": np.zeros((PL,), np.float32),
    }
    out = kernel(**ins)
    print("out", out.shape, out.dtype, float(np.abs(out).max()))
